# revision 26
# baseline (speedup 1.0000x reference)
"""MoE (E=8 experts, top-2, D=1024, T=8192) — expert-parallel Trainium2 kernel.

Strategy (per the expert-parallel sharding hint):
  - Host computes the gate (0.1% of FLOPs: scores, top-2, softmax) and uses it
    to shard tokens: each of the 8 NeuronCores owns one expert and receives
    exactly the tokens routed to it (padded to a common capacity C).
  - Each core runs the dense expert FFN + layernorm + combine-weight scaling
    over its routed tokens: 99.9% of the FLOPs.
  - Host gathers the per-expert outputs back into token order (pure gather —
    slot assignment makes a scatter unnecessary) and sums the K=2 contributions.

Device dataflow (activations kept transposed, features on partitions, so the
mm1 -> gelu -> mm2 chain composes with zero transposes):
  xT [D, C] --mm1--> hT [2D, C] --gelu+b1--> mm2 --> + x residual (f32 stream,
  DVE add) --> zT [D, C] --LN scale--> z*(wv*rstd) --> outT

The FFN matmuls run in fp8-e4m3 with perf_mode=DoubleRow (2 MACs/cell/cycle:
the PE virtualizes to 128x256, halving matmul instruction count vs bf16).
Weights are host-prescaled by 2^10 (and x by 2^4 for mm1) so the e4m3 lattice
is used well clear of denormals; the gelu activation unscales mm1 by 2^-14.
mm2's 2^10 output scale is never unscaled: the residual stream arrives as
x*2^10 and layernorm is scale-invariant (eps is scaled by 2^20 to match), so
z' = 2^10*z flows straight through the stats and the final scale.

The LN mean term is applied on the HOST: out = wv*rstd*(z - mean) splits into
wv*rstd*z (device) minus (wv*rstd*mean) (a per-token scalar the device ships
as a 1-row tensor), so the device normalize is one DVE multiply per chunk
instead of subtract+multiply.

LN statistics are computed on the PE: an accumulating ones[128,128].T @ zT
matmul (f32r - fp32 truncated to fp22, full PE rate at >=256 cols) sums over
the feature (partition) axis AND broadcasts across all 128 partitions in one
instruction. w1/w2 are fully SBUF-resident in fp8 (4 MB), host-relaid so every
DMA lands as one fat contiguous descriptor per partition (the [m, ko, col]
nesting matches the lhsT access pattern). mm1 is software-pipelined one tile
ahead of mm2/LN, with mm2 accumulation groups interleaved between mm1 groups
so the in-order PE always has an independent group queued behind any group
waiting on DMA or the DVE.
"""

import sys

sys.path.insert(0, "/opt/trn_rl_repo")

import ml_dtypes
import numpy as np

E, K, D = 8, 2, 1024
H = 2 * D  # ffn hidden
B, S = 2, 4096
T = B * S
CT = 512  # max token tile (PSUM bank = 512 f32)
CT_MIN = 128  # remainder-tile granularity
P = 128
KO1 = D // P  # 8  k-chunks for mm1 (4 DoubleRow pairs)
MO1 = H // P  # 16 m-chunks for mm1
KO2 = H // P  # 16 k-chunks for mm2 (8 DoubleRow pairs)
MO2 = D // P  # 8  m-chunks for mm2

SW = 1024.0  # weight prescale (2^10)
SX = 16.0  # x prescale for the fp8 mm1 moving operand (2^4)
FP8_MAX = 240.0  # TRN float8e4 max normal

_kernel_cache = {}


def _tiles_for(C):
    tiles = [(i * CT, CT) for i in range(C // CT)]
    if C % CT:
        tiles.append((C - C % CT, C % CT))
    return tiles


def _build_bass(C, use_b2, use_lng, use_lnb):
    """Build the per-core expert-FFN kernel for capacity C (multiple of CT_MIN)."""
    import concourse.tile as tile
    from concourse import bacc, mybir

    f32 = mybir.dt.float32
    f32r = mybir.dt.float32r
    fp8 = mybir.dt.float8e4
    AF = mybir.ActivationFunctionType
    OP = mybir.AluOpType
    DR = mybir.MatmulPerfMode.DoubleRow

    assert C % CT_MIN == 0
    tiles = _tiles_for(C)
    NT = len(tiles)

    nc = bacc.Bacc("TRN2", target_bir_lowering=False, debug=False)
    # xT/xres ship in per-tile-padded block layouts so each tile's DMA is one
    # contiguous descriptor per partition
    xT_d = nc.dram_tensor("xT", [P, NT, KO1, CT], fp8, kind="ExternalInput")
    xTf_d = nc.dram_tensor("xTf", [P, NT, MO2, CT], f32r, kind="ExternalInput")
    w1_d = nc.dram_tensor("w1", [P, MO1, KO1, P], fp8, kind="ExternalInput")
    w2_d = nc.dram_tensor("w2", [P, MO2, KO2, P], fp8, kind="ExternalInput")
    # host pre-transposes the per-feature constants to [P, mo] so each is a
    # single fat DMA descriptor instead of ~2k 4-byte gathers
    b1_d = nc.dram_tensor("b1", [P, MO1], f32, kind="ExternalInput")
    b2_d = nc.dram_tensor("b2", [P, MO2], f32, kind="ExternalInput")
    lng_d = nc.dram_tensor("ln_g", [P, MO2], f32, kind="ExternalInput")
    lnb_d = nc.dram_tensor("ln_b", [P, MO2], f32, kind="ExternalInput")
    wv_d = nc.dram_tensor("wv", [P, C], f32, kind="ExternalInput")
    out_d = nc.dram_tensor("outT", [D, C], f32, kind="ExternalOutput")
    outS_d = nc.dram_tensor("outS", [1, C], f32, kind="ExternalOutput")

    out_r = out_d.rearrange("(mo p) c -> p mo c", p=P)

    with tile.TileContext(nc) as tc:
        with (
            tc.tile_pool(name="singles", bufs=1) as singles,
            tc.tile_pool(name="xp", bufs=3) as xp,
            tc.tile_pool(name="hp", bufs=2) as hp,
            tc.tile_pool(name="zp", bufs=2) as zp,
            tc.tile_pool(name="zqp", bufs=6) as zqp,
            tc.tile_pool(name="wvp", bufs=3) as wvp,
            tc.tile_pool(name="xrp", bufs=2) as xrp,
            tc.tile_pool(name="stp", bufs=3) as stp,
            tc.tile_pool(name="ocp", bufs=6) as ocp,
            tc.tile_pool(name="psmm", bufs=6, space="PSUM") as psmm,
            tc.tile_pool(name="psst", bufs=1, space="PSUM") as psst,
        ):
            # ---- resident data ----
            # tile 0's xT first (SP ring is FIFO), then w1 in 8 chunks
            # alternating across both HWDGE rings so tile 0's matmuls never
            # outrun the weight stream; w2 is deferred to mid-tile-0 emission
            # so it does not steal HBM bandwidth from w1
            xT_tiles = [xp.tile([P, KO1, CT], fp8, name="xT_sb") for _ in range(3)]
            nc.sync.dma_start(xT_tiles[0][:], xT_d[:, 0])
            ones_f32 = singles.tile([P, P], f32)
            nc.vector.memset(ones_f32[:], 1.0)
            ones_sb = singles.tile([P, P], f32r)
            nc.vector.tensor_copy(ones_sb[:], ones_f32[:])
            eps_sb = singles.tile([P, 1], f32)
            nc.vector.memset(eps_sb[:], 1e-6 * SW * SW)  # eps on 2^10-scaled z
            # each weight chunk is its own tile: the tile dep-tracker is
            # per-tile, so a single resident tile would stall the first matmul
            # until every chunk DMA lands
            w1_ch = [singles.tile([P, 2, KO1, P], fp8, name=f"w1c{i}") for i in range(MO1 // 2)]
            for mq in range(MO1 // 2):
                eng = nc.sync if mq % 2 == 0 else nc.scalar
                eng.dma_start(w1_ch[mq][:], w1_d[:, 2 * mq : 2 * mq + 2])
            b1_sb = singles.tile([P, MO1], f32)
            nc.scalar.dma_start(b1_sb[:], b1_d[:])
            b2_sb = singles.tile([P, MO2], f32)
            nc.scalar.dma_start(b2_sb[:], b2_d[:])
            lng_sb = singles.tile([P, MO2], f32)
            nc.scalar.dma_start(lng_sb[:], lng_d[:])
            lnb_sb = singles.tile([P, MO2], f32)
            nc.scalar.dma_start(lnb_sb[:], lnb_d[:])
            w2_ch = [singles.tile([P, 2, KO2, P], fp8, name=f"w2c{i}") for i in range(MO2 // 2)]

            def emit_w2_chunk(mq):
                nc.scalar.dma_start(w2_ch[mq][:], w2_d[:, 2 * mq : 2 * mq + 2])

            def emit_mm1(t, t0, ct, xT_sb, st, prev_st=None):
                """mm1 (fp8 DoubleRow) + gelu into a fresh hT tile; mm2 groups
                of the previous tile interleave so the PE always has an
                independent accumulation group queued behind a stalled one."""
                hT_sb = hp.tile([P, KO2, CT], fp8, name="hT_sb")
                for m in range(MO1):
                    ps = psmm.tile([P, CT], f32, name="ps_mm")
                    for k in range(KO1 // 2):
                        nc.tensor.matmul(
                            ps[:, :ct],
                            w1_ch[m // 2][:, m % 2, 2 * k : 2 * k + 2, :],
                            xT_sb[:, 2 * k : 2 * k + 2, :ct],
                            start=(k == 0),
                            stop=(k == KO1 // 2 - 1),
                            perf_mode=DR,
                        )
                    # gelu(2^-14 * ps + b1) -> fp8 h (unscaled)
                    nc.scalar.activation(
                        out=hT_sb[:, m, :ct],
                        in_=ps[:, :ct],
                        func=AF.Gelu,
                        bias=b1_sb[:, m : m + 1],
                        scale=1.0 / (SW * SX),
                    )
                    if t == 0 and m in (4, 6, 8, 10):
                        emit_w2_chunk((m - 4) // 2)
                    if m == (6 if t == 0 else 2):
                        nc.scalar.dma_start(st["wv_sb"], wv_d[:, st["ts"]])
                        nc.scalar.dma_start(st["xres"][:, 0:4], xTf_d[:, t, 0:4])
                        nc.scalar.dma_start(st["xres"][:, 4:8], xTf_d[:, t, 4:8])
                    if prev_st is not None and m % 2 == 1:
                        mm2_group(prev_st, m // 2)
                st["hT_sb"] = hT_sb
                return hT_sb

            def mm2_state(t, t0, ct):
                """Allocated ahead of mm1(t); the xres/wv DMAs are emitted
                inside emit_mm1(t) behind a few of tile t's gelus, so they
                start a full tile before mm2(t) consumes them but never steal
                HBM bandwidth from the startup weight stream."""
                ts = slice(t0, t0 + ct)
                wv_sb = wvp.tile([P, CT], f32, name="wv_sb")[:, :ct]
                xres = xrp.tile([P, MO2, CT], f32r, name="xres")
                return {
                    "t": t, "t0": t0, "ct": ct, "ts": ts, "hT_sb": None,
                    "wv_sb": wv_sb, "xres": xres, "norm_prev": None,
                    "zT_sb": zp.tile([P, MO2, CT], f32r, name="zT_sb"),
                    "ps_S": psst.tile([P, CT], f32, name="ps_S"),
                    "ps_Q": psst.tile([P, CT], f32, name="ps_Q"),
                    "pending": [],
                }

            def mm2_stats(st, mo, zc, zq):
                ct = st["ct"]
                nc.tensor.matmul(
                    st["ps_S"][:, :ct], ones_sb[:], zc,
                    start=(mo == 0), stop=(mo == MO2 - 1), skip_group_check=True,
                )
                nc.tensor.matmul(
                    st["ps_Q"][:, :ct], ones_sb[:], zq,
                    start=(mo == 0), stop=(mo == MO2 - 1), skip_group_check=True,
                )

            def mm2_group(st, mo):
                ct = st["ct"]
                ps = psmm.tile([P, CT], f32, name="ps_mm")
                for k in range(KO2 // 2):
                    nc.tensor.matmul(
                        ps[:, :ct],
                        w2_ch[mo // 2][:, mo % 2, 2 * k : 2 * k + 2, :],
                        st["hT_sb"][:, 2 * k : 2 * k + 2, :ct],
                        start=(k == 0),
                        stop=(k == KO2 // 2 - 1),
                        perf_mode=DR,
                    )
                if st["pending"]:
                    mm2_stats(st, *st["pending"].pop())
                zc = st["zT_sb"][:, mo, :ct]
                nc.vector.tensor_tensor(zc, ps[:, :ct], st["xres"][:, mo, :ct], OP.add)
                if use_b2:
                    nc.vector.tensor_scalar_add(zc, zc, b2_sb[:, mo : mo + 1])
                zq = zqp.tile([P, CT], f32r, name="zq")[:, :ct]
                nc.vector.tensor_tensor(zq, zc, zc, OP.mult)
                st["pending"].append((mo, zc, zq))
                # the previous tile's normalize rides one chunk at a time
                # behind this tile's z chain, so the in-order DVE never delays
                # the stats matmuls by a whole tile's worth of normalize work
                if st["norm_prev"] is not None:
                    ln_norm_chunk(st["norm_prev"], mo)

            def ln_stats(st):
                ct, ts, wv_sb = st["ct"], st["ts"], st["wv_sb"]
                mm2_stats(st, *st["pending"].pop())
                # LN stats: mean/rstd replicated across partitions
                mean_sb = stp.tile([P, CT], f32, name="mean")[:, :ct]
                nc.vector.tensor_scalar_mul(mean_sb, st["ps_S"][:, :ct], 1.0 / D)
                rw_sb = stp.tile([P, CT], f32, name="rw")[:, :ct]
                nc.vector.tensor_scalar_mul(rw_sb, st["ps_Q"][:, :ct], 1.0 / D)
                msq_sb = zqp.tile([P, CT], f32, name="zq")[:, :ct]
                nc.vector.tensor_tensor(msq_sb, mean_sb, mean_sb, OP.mult)
                nc.vector.tensor_tensor(rw_sb, rw_sb, msq_sb, OP.subtract)
                # rw = wv / sqrt(var + eps): sqrt on ACT, fast-reciprocal on DVE
                nc.scalar.activation(
                    out=rw_sb, in_=rw_sb, func=AF.Sqrt, bias=eps_sb[:], scale=1.0,
                )
                nc.vector.reciprocal_approx_fast(rw_sb, rw_sb)
                nc.vector.tensor_tensor(rw_sb, rw_sb, wv_sb, OP.mult)
                # per-token mean term, applied host-side after the combine
                mrw_sb = zqp.tile([P, CT], f32, name="zq")[:, :ct]
                nc.vector.tensor_tensor(mrw_sb, mean_sb, rw_sb, OP.mult)
                nc.scalar.dma_start(outS_d[0:1, ts], mrw_sb[0:1, :ct])
                st["rw_sb"] = rw_sb

            def ln_norm_chunk(st, mo):
                ct = st["ct"]
                occ = ocp.tile([P, CT], f32, name="oc")[:, :ct]
                nc.gpsimd.tensor_tensor(occ, st["zT_sb"][:, mo, :ct], st["rw_sb"], OP.mult)
                if use_lng:
                    nc.gpsimd.tensor_scalar_mul(occ, occ, lng_sb[:, mo : mo + 1])
                if use_lnb:
                    lb = stp.tile([P, CT], f32, name="lb")[:, :ct]
                    nc.gpsimd.tensor_scalar_mul(lb, st["wv_sb"], lnb_sb[:, mo : mo + 1])
                    nc.gpsimd.tensor_tensor(occ, occ, lb, OP.add)
                nc.sync.dma_start(out_r[:, mo, st["ts"]], occ)

            # xT(t) is consumed only by mm1(t) (residual uses the xres
            # stream); prefetches are queued after the w1 chunks so they never
            # delay the critical w1 stream
            prev_st = None
            for t, (t0, ct) in enumerate(tiles):
                st = mm2_state(t, t0, ct)
                emit_mm1(t, t0, ct, xT_tiles[t % 3], st, prev_st)
                for tn_i in ([1, 2] if t == 0 else [t + 2]):
                    if tn_i < NT:
                        nc.sync.dma_start(xT_tiles[tn_i % 3][:], xT_d[:, tn_i])
                if prev_st is not None:
                    ln_stats(prev_st)
                    st["norm_prev"] = prev_st
                prev_st = st
            for mo in range(MO2):
                mm2_group(prev_st, mo)
            ln_stats(prev_st)
            for mo in range(MO2):
                ln_norm_chunk(prev_st, mo)

    nc.finalize()
    return nc


def _route(x, gate_w):
    """Host gate: top-2 per token + softmax combine weights (matches
    jax.lax.top_k tie-breaking: lower index wins)."""
    xt = x.reshape(-1, D)
    scores = xt.astype(np.float32) @ gate_w.astype(np.float32)  # [T, E]
    e0 = np.argmax(scores, axis=1)
    s0 = scores[np.arange(T), e0]
    masked = scores.copy()
    masked[np.arange(T), e0] = -np.inf
    e1 = np.argmax(masked, axis=1)
    s1 = masked[np.arange(T), e1]
    # softmax over the two selected scores
    mx = np.maximum(s0, s1)
    z0 = np.exp((s0 - mx).astype(np.float64))
    z1 = np.exp((s1 - mx).astype(np.float64))
    den = z0 + z1
    w0 = (z0 / den).astype(np.float32)
    w1 = (z1 / den).astype(np.float32)
    return xt, e0, e1, w0, w1


def _fp8(a, scale):
    return np.clip(a * scale, -FP8_MAX, FP8_MAX).astype(ml_dtypes.float8_e4m3)


def _wlay(w, ko, mo):
    """[ko*P, mo*P] -> [P, mo, ko, P]: partition-contiguous DMA layout that
    matches the lhsT access pattern w_sb[:, m, 2k:2k+2, :]."""
    return np.ascontiguousarray(
        np.asarray(w).reshape(ko, P, mo, P).transpose(1, 2, 0, 3)
    )


def kernel(x, gate_w, w1, b1, w2, b2, ln_g, ln_b):
    from concourse.bass_utils import run_bass_kernel_spmd

    x = np.asarray(x)
    xt, e0, e1, wk0, wk1 = _route(x, np.asarray(gate_w))

    # slot assignment: expert e's token list = tokens with e0==e, then e1==e
    idx_e, wv_e = [], []
    for e in range(E):
        i0 = np.nonzero(e0 == e)[0]
        i1 = np.nonzero(e1 == e)[0]
        idx_e.append(np.concatenate([i0, i1]))
        wv_e.append(np.concatenate([wk0[i0], wk1[i1]]))
    maxn = max(len(i) for i in idx_e)
    C = max(CT_MIN, -(-maxn // CT_MIN) * CT_MIN)
    tiles = _tiles_for(C)
    NT = len(tiles)

    use_b2 = bool(np.any(np.asarray(b2) != 0))
    use_lng = bool(np.any(np.asarray(ln_g) != 1))
    use_lnb = bool(np.any(np.asarray(ln_b) != 0))
    key = (C, use_b2, use_lng, use_lnb)
    if key not in _kernel_cache:
        _kernel_cache[key] = _build_bass(C, use_b2, use_lng, use_lnb)
    nc = _kernel_cache[key]

    in_maps = []
    for e in range(E):
        n = len(idx_e[e])
        xTe = np.zeros((D, C), np.float32)
        xTe[:, :n] = xt[idx_e[e]].T
        wve = np.zeros((C,), np.float32)
        wve[:n] = wv_e[e]
        # per-tile padded block layouts: [P, NT, KO1|MO2, CT]
        x8 = _fp8(xTe, SX).reshape(KO1, P, C)
        xr = (xTe * SW).reshape(MO2, P, C)
        xT_blk = np.zeros((P, NT, KO1, CT), ml_dtypes.float8_e4m3)
        xr_blk = np.zeros((P, NT, MO2, CT), np.float32)
        for t, (t0, ct) in enumerate(tiles):
            xT_blk[:, t, :, :ct] = x8[:, :, t0 : t0 + ct].transpose(1, 0, 2)
            xr_blk[:, t, :, :ct] = xr[:, :, t0 : t0 + ct].transpose(1, 0, 2)
        in_maps.append({
            "xT": xT_blk,
            "xTf": xr_blk,  # 2^10-scaled residual (mm2's never-unscaled scale)
            "w1": _fp8(_wlay(w1[e], KO1, MO1), SW),
            "w2": _fp8(_wlay(w2[e], KO2, MO2), SW),
            "b1": np.ascontiguousarray(np.asarray(b1)[e].reshape(MO1, P).T),
            "b2": np.ascontiguousarray((np.asarray(b2)[e] * SW).reshape(MO2, P).T),
            "ln_g": np.ascontiguousarray(np.asarray(ln_g)[e].reshape(MO2, P).T),
            "ln_b": np.ascontiguousarray(np.asarray(ln_b)[e].reshape(MO2, P).T),
            "wv": np.broadcast_to(wve, (P, C)).copy(),
        })

    res = run_bass_kernel_spmd(nc, in_maps, core_ids=list(range(E)))
    kernel.last_results = res

    # combine: token t's two contributions live at known (expert, slot) pairs
    slot0 = np.empty(T, np.int64)
    slot1 = np.empty(T, np.int64)
    for e in range(E):
        n0 = int(np.sum(e0 == e))
        slot0[e0 == e] = np.arange(n0)
        slot1[e1 == e] = n0 + np.arange(int(np.sum(e1 == e)))
    Y = np.stack([res.results[e]["outT"] for e in range(E)])  # [E, D, C]
    Sm = np.stack([res.results[e]["outS"][0] for e in range(E)])  # [E, C]
    out = Y[e0, :, slot0] + Y[e1, :, slot1]  # [T, D]
    # host-side LN mean term: out -= sum_e (wv*rstd*mean)_e ⊗ ln_g_e
    lng = np.asarray(ln_g, np.float32)
    if use_lng:
        out -= Sm[e0, slot0][:, None] * lng[e0] + Sm[e1, slot1][:, None] * lng[e1]
    else:
        out -= (Sm[e0, slot0] + Sm[e1, slot1])[:, None]
    return out.reshape(x.shape).astype(np.float32)


# revision 27
# speedup vs baseline: 1.0004x; 1.0004x over previous
"""MoE (E=8 experts, top-2, D=1024, T=8192) — expert-parallel Trainium2 kernel.

Strategy (per the expert-parallel sharding hint):
  - Host computes the gate (0.1% of FLOPs: scores, top-2, softmax) and uses it
    to shard tokens: each of the 8 NeuronCores owns one expert and receives
    exactly the tokens routed to it (padded to a common capacity C).
  - Each core runs the dense expert FFN + layernorm + combine-weight scaling
    over its routed tokens: 99.9% of the FLOPs.
  - Host gathers the per-expert outputs back into token order (pure gather —
    slot assignment makes a scatter unnecessary) and sums the K=2 contributions.

Device dataflow (activations kept transposed, features on partitions, so the
mm1 -> gelu -> mm2 chain composes with zero transposes):
  xT [D, C] --mm1--> hT [2D, C] --gelu+b1--> mm2 --> + x residual (f32 stream,
  DVE add) --> zT [D, C] --LN scale--> z*(wv*rstd) --> outT

The FFN matmuls run in fp8-e4m3 with perf_mode=DoubleRow (2 MACs/cell/cycle:
the PE virtualizes to 128x256, halving matmul instruction count vs bf16).
Weights are host-prescaled by 2^10 (and x by 2^4 for mm1) so the e4m3 lattice
is used well clear of denormals; the gelu activation unscales mm1 by 2^-14.
mm2's 2^10 output scale is never unscaled: the residual stream arrives as
x*2^10 and layernorm is scale-invariant (eps is scaled by 2^20 to match), so
z' = 2^10*z flows straight through the stats and the final scale.

The LN mean term is applied on the HOST: out = wv*rstd*(z - mean) splits into
wv*rstd*z (device) minus (wv*rstd*mean) (a per-token scalar the device ships
as a 1-row tensor), so the device normalize is one DVE multiply per chunk
instead of subtract+multiply.

LN statistics are computed on the PE: an accumulating ones[128,128].T @ zT
matmul (f32r - fp32 truncated to fp22, full PE rate at >=256 cols) sums over
the feature (partition) axis AND broadcasts across all 128 partitions in one
instruction. w1/w2 are fully SBUF-resident in fp8 (4 MB), host-relaid so every
DMA lands as one fat contiguous descriptor per partition (the [m, ko, col]
nesting matches the lhsT access pattern). mm1 is software-pipelined one tile
ahead of mm2/LN, with mm2 accumulation groups interleaved between mm1 groups
so the in-order PE always has an independent group queued behind any group
waiting on DMA or the DVE.
"""

import sys

sys.path.insert(0, "/opt/trn_rl_repo")

import ml_dtypes
import numpy as np

E, K, D = 8, 2, 1024
H = 2 * D  # ffn hidden
B, S = 2, 4096
T = B * S
CT = 512  # max token tile (PSUM bank = 512 f32)
CT_MIN = 128  # remainder-tile granularity
P = 128
KO1 = D // P  # 8  k-chunks for mm1 (4 DoubleRow pairs)
MO1 = H // P  # 16 m-chunks for mm1
KO2 = H // P  # 16 k-chunks for mm2 (8 DoubleRow pairs)
MO2 = D // P  # 8  m-chunks for mm2

SW = 1024.0  # weight prescale (2^10)
SX = 16.0  # x prescale for the fp8 mm1 moving operand (2^4)
FP8_MAX = 240.0  # TRN float8e4 max normal

_kernel_cache = {}


def _tiles_for(C):
    tiles = [(i * CT, CT) for i in range(C // CT)]
    if C % CT:
        tiles.append((C - C % CT, C % CT))
    return tiles


def _build_bass(C, use_b2, use_lng, use_lnb):
    """Build the per-core expert-FFN kernel for capacity C (multiple of CT_MIN)."""
    import concourse.tile as tile
    from concourse import bacc, mybir

    f32 = mybir.dt.float32
    f32r = mybir.dt.float32r
    fp8 = mybir.dt.float8e4
    AF = mybir.ActivationFunctionType
    OP = mybir.AluOpType
    DR = mybir.MatmulPerfMode.DoubleRow

    assert C % CT_MIN == 0
    tiles = _tiles_for(C)
    NT = len(tiles)

    nc = bacc.Bacc("TRN2", target_bir_lowering=False, debug=False)
    # xT/xres ship in per-tile-padded block layouts so each tile's DMA is one
    # contiguous descriptor per partition
    xT_d = nc.dram_tensor("xT", [P, NT, KO1, CT], fp8, kind="ExternalInput")
    xTf_d = nc.dram_tensor("xTf", [P, NT, MO2, CT], f32r, kind="ExternalInput")
    w1_d = nc.dram_tensor("w1", [P, MO1, KO1, P], fp8, kind="ExternalInput")
    w2_d = nc.dram_tensor("w2", [P, MO2, KO2, P], fp8, kind="ExternalInput")
    # host pre-transposes the per-feature constants to [P, mo] so each is a
    # single fat DMA descriptor instead of ~2k 4-byte gathers
    b1_d = nc.dram_tensor("b1", [P, MO1], f32, kind="ExternalInput")
    b2_d = nc.dram_tensor("b2", [P, MO2], f32, kind="ExternalInput")
    lng_d = nc.dram_tensor("ln_g", [P, MO2], f32, kind="ExternalInput")
    lnb_d = nc.dram_tensor("ln_b", [P, MO2], f32, kind="ExternalInput")
    wv_d = nc.dram_tensor("wv", [P, C], f32, kind="ExternalInput")
    out_d = nc.dram_tensor("outT", [D, C], f32, kind="ExternalOutput")
    outS_d = nc.dram_tensor("outS", [1, C], f32, kind="ExternalOutput")

    out_r = out_d.rearrange("(mo p) c -> p mo c", p=P)

    with tile.TileContext(nc) as tc:
        with (
            tc.tile_pool(name="singles", bufs=1) as singles,
            tc.tile_pool(name="xp", bufs=3) as xp,
            tc.tile_pool(name="hp", bufs=2) as hp,
            tc.tile_pool(name="zp", bufs=3) as zp,
            tc.tile_pool(name="zqp", bufs=6) as zqp,
            tc.tile_pool(name="wvp", bufs=3) as wvp,
            tc.tile_pool(name="xrp", bufs=2) as xrp,
            tc.tile_pool(name="stp", bufs=3) as stp,
            tc.tile_pool(name="ocp", bufs=6) as ocp,
            tc.tile_pool(name="psmm", bufs=6, space="PSUM") as psmm,
            tc.tile_pool(name="psst", bufs=1, space="PSUM") as psst,
        ):
            # ---- resident data ----
            # tile 0's xT first (SP ring is FIFO), then w1 in 8 chunks
            # alternating across both HWDGE rings so tile 0's matmuls never
            # outrun the weight stream; w2 is deferred to mid-tile-0 emission
            # so it does not steal HBM bandwidth from w1
            xT_tiles = [xp.tile([P, KO1, CT], fp8, name="xT_sb") for _ in range(3)]
            nc.sync.dma_start(xT_tiles[0][:], xT_d[:, 0])
            ones_f32 = singles.tile([P, P], f32)
            nc.vector.memset(ones_f32[:], 1.0)
            ones_sb = singles.tile([P, P], f32r)
            nc.vector.tensor_copy(ones_sb[:], ones_f32[:])
            eps_sb = singles.tile([P, 1], f32)
            nc.vector.memset(eps_sb[:], 1e-6 * SW * SW)  # eps on 2^10-scaled z
            # each weight chunk is its own tile: the tile dep-tracker is
            # per-tile, so a single resident tile would stall the first matmul
            # until every chunk DMA lands
            w1_ch = [singles.tile([P, 2, KO1, P], fp8, name=f"w1c{i}") for i in range(MO1 // 2)]
            for mq in range(MO1 // 2):
                eng = nc.sync if mq % 2 == 0 else nc.scalar
                eng.dma_start(w1_ch[mq][:], w1_d[:, 2 * mq : 2 * mq + 2])
            b1_sb = singles.tile([P, MO1], f32)
            nc.scalar.dma_start(b1_sb[:], b1_d[:])
            b2_sb = singles.tile([P, MO2], f32)
            nc.scalar.dma_start(b2_sb[:], b2_d[:])
            lng_sb = singles.tile([P, MO2], f32)
            nc.scalar.dma_start(lng_sb[:], lng_d[:])
            lnb_sb = singles.tile([P, MO2], f32)
            nc.scalar.dma_start(lnb_sb[:], lnb_d[:])
            w2_ch = [singles.tile([P, 2, KO2, P], fp8, name=f"w2c{i}") for i in range(MO2 // 2)]

            def emit_w2_chunk(mq):
                nc.scalar.dma_start(w2_ch[mq][:], w2_d[:, 2 * mq : 2 * mq + 2])

            def emit_mm1(t, t0, ct, xT_sb, st, prev_st=None):
                """mm1 (fp8 DoubleRow) + gelu into a fresh hT tile; mm2 groups
                of the previous tile interleave so the PE always has an
                independent accumulation group queued behind a stalled one."""
                hT_sb = hp.tile([P, KO2, CT], fp8, name="hT_sb")
                for m in range(MO1):
                    ps = psmm.tile([P, CT], f32, name="ps_mm")
                    for k in range(KO1 // 2):
                        nc.tensor.matmul(
                            ps[:, :ct],
                            w1_ch[m // 2][:, m % 2, 2 * k : 2 * k + 2, :],
                            xT_sb[:, 2 * k : 2 * k + 2, :ct],
                            start=(k == 0),
                            stop=(k == KO1 // 2 - 1),
                            perf_mode=DR,
                        )
                    # gelu(2^-14 * ps + b1) -> fp8 h (unscaled)
                    nc.scalar.activation(
                        out=hT_sb[:, m, :ct],
                        in_=ps[:, :ct],
                        func=AF.Gelu,
                        bias=b1_sb[:, m : m + 1],
                        scale=1.0 / (SW * SX),
                    )
                    if t == 0 and m in (4, 6, 8, 10):
                        emit_w2_chunk((m - 4) // 2)
                    if m == (6 if t == 0 else 2):
                        nc.scalar.dma_start(st["wv_sb"], wv_d[:, st["ts"]])
                        nc.scalar.dma_start(st["xres"][:, 0:4], xTf_d[:, t, 0:4])
                        nc.scalar.dma_start(st["xres"][:, 4:8], xTf_d[:, t, 4:8])
                    if prev_st is not None and m % 2 == 1:
                        mm2_group(prev_st, m // 2)
                st["hT_sb"] = hT_sb
                return hT_sb

            def mm2_state(t, t0, ct):
                """Allocated ahead of mm1(t); the xres/wv DMAs are emitted
                inside emit_mm1(t) behind a few of tile t's gelus, so they
                start a full tile before mm2(t) consumes them but never steal
                HBM bandwidth from the startup weight stream."""
                ts = slice(t0, t0 + ct)
                wv_sb = wvp.tile([P, CT], f32, name="wv_sb")[:, :ct]
                xres = xrp.tile([P, MO2, CT], f32r, name="xres")
                return {
                    "t": t, "t0": t0, "ct": ct, "ts": ts, "hT_sb": None,
                    "wv_sb": wv_sb, "xres": xres, "norm_prev": None,
                    "zT_sb": zp.tile([P, MO2, CT], f32r, name="zT_sb"),
                    "ps_S": psst.tile([P, CT], f32, name="ps_S"),
                    "ps_Q": psst.tile([P, CT], f32, name="ps_Q"),
                    "pending": [],
                }

            def mm2_stats(st, mo, zc, zq):
                ct = st["ct"]
                nc.tensor.matmul(
                    st["ps_S"][:, :ct], ones_sb[:], zc,
                    start=(mo == 0), stop=(mo == MO2 - 1), skip_group_check=True,
                )
                nc.tensor.matmul(
                    st["ps_Q"][:, :ct], ones_sb[:], zq,
                    start=(mo == 0), stop=(mo == MO2 - 1), skip_group_check=True,
                )

            def mm2_group(st, mo):
                ct = st["ct"]
                ps = psmm.tile([P, CT], f32, name="ps_mm")
                for k in range(KO2 // 2):
                    nc.tensor.matmul(
                        ps[:, :ct],
                        w2_ch[mo // 2][:, mo % 2, 2 * k : 2 * k + 2, :],
                        st["hT_sb"][:, 2 * k : 2 * k + 2, :ct],
                        start=(k == 0),
                        stop=(k == KO2 // 2 - 1),
                        perf_mode=DR,
                    )
                if st["pending"]:
                    mm2_stats(st, *st["pending"].pop())
                zc = st["zT_sb"][:, mo, :ct]
                nc.vector.tensor_tensor(zc, ps[:, :ct], st["xres"][:, mo, :ct], OP.add)
                if use_b2:
                    nc.vector.tensor_scalar_add(zc, zc, b2_sb[:, mo : mo + 1])
                zq = zqp.tile([P, CT], f32r, name="zq")[:, :ct]
                nc.vector.tensor_tensor(zq, zc, zc, OP.mult)
                st["pending"].append((mo, zc, zq))
                # the previous tile's normalize rides one chunk at a time
                # behind this tile's z chain, so the in-order DVE never delays
                # the stats matmuls by a whole tile's worth of normalize work
                if st["norm_prev"] is not None:
                    ln_norm_chunk(st["norm_prev"], mo)

            def ln_stats(st):
                ct, ts, wv_sb = st["ct"], st["ts"], st["wv_sb"]
                mm2_stats(st, *st["pending"].pop())
                # LN stats: mean/rstd replicated across partitions
                mean_sb = stp.tile([P, CT], f32, name="mean")[:, :ct]
                nc.vector.tensor_scalar_mul(mean_sb, st["ps_S"][:, :ct], 1.0 / D)
                rw_sb = stp.tile([P, CT], f32, name="rw")[:, :ct]
                nc.vector.tensor_scalar_mul(rw_sb, st["ps_Q"][:, :ct], 1.0 / D)
                msq_sb = zqp.tile([P, CT], f32, name="zq")[:, :ct]
                nc.vector.tensor_tensor(msq_sb, mean_sb, mean_sb, OP.mult)
                nc.vector.tensor_tensor(rw_sb, rw_sb, msq_sb, OP.subtract)
                # rw = wv / sqrt(var + eps): sqrt on ACT, fast-reciprocal on DVE
                nc.scalar.activation(
                    out=rw_sb, in_=rw_sb, func=AF.Sqrt, bias=eps_sb[:], scale=1.0,
                )
                nc.vector.reciprocal_approx_fast(rw_sb, rw_sb)
                nc.vector.tensor_tensor(rw_sb, rw_sb, wv_sb, OP.mult)
                # per-token mean term, applied host-side after the combine
                mrw_sb = zqp.tile([P, CT], f32, name="zq")[:, :ct]
                nc.vector.tensor_tensor(mrw_sb, mean_sb, rw_sb, OP.mult)
                nc.scalar.dma_start(outS_d[0:1, ts], mrw_sb[0:1, :ct])
                st["rw_sb"] = rw_sb

            def ln_norm_chunk(st, mo):
                ct = st["ct"]
                occ = ocp.tile([P, CT], f32, name="oc")[:, :ct]
                nc.gpsimd.tensor_tensor(occ, st["zT_sb"][:, mo, :ct], st["rw_sb"], OP.mult)
                if use_lng:
                    nc.gpsimd.tensor_scalar_mul(occ, occ, lng_sb[:, mo : mo + 1])
                if use_lnb:
                    lb = stp.tile([P, CT], f32, name="lb")[:, :ct]
                    nc.gpsimd.tensor_scalar_mul(lb, st["wv_sb"], lnb_sb[:, mo : mo + 1])
                    nc.gpsimd.tensor_tensor(occ, occ, lb, OP.add)
                nc.sync.dma_start(out_r[:, mo, st["ts"]], occ)

            # xT(t) is consumed only by mm1(t) (residual uses the xres
            # stream); prefetches are queued after the w1 chunks so they never
            # delay the critical w1 stream
            prev_st = None
            for t, (t0, ct) in enumerate(tiles):
                st = mm2_state(t, t0, ct)
                emit_mm1(t, t0, ct, xT_tiles[t % 3], st, prev_st)
                for tn_i in ([1, 2] if t == 0 else [t + 2]):
                    if tn_i < NT:
                        nc.sync.dma_start(xT_tiles[tn_i % 3][:], xT_d[:, tn_i])
                if prev_st is not None:
                    ln_stats(prev_st)
                    st["norm_prev"] = prev_st
                prev_st = st
            for mo in range(MO2):
                mm2_group(prev_st, mo)
            ln_stats(prev_st)
            for mo in range(MO2):
                ln_norm_chunk(prev_st, mo)

    nc.finalize()
    return nc


def _route(x, gate_w):
    """Host gate: top-2 per token + softmax combine weights (matches
    jax.lax.top_k tie-breaking: lower index wins)."""
    xt = x.reshape(-1, D)
    scores = xt.astype(np.float32) @ gate_w.astype(np.float32)  # [T, E]
    e0 = np.argmax(scores, axis=1)
    s0 = scores[np.arange(T), e0]
    masked = scores.copy()
    masked[np.arange(T), e0] = -np.inf
    e1 = np.argmax(masked, axis=1)
    s1 = masked[np.arange(T), e1]
    # softmax over the two selected scores
    mx = np.maximum(s0, s1)
    z0 = np.exp((s0 - mx).astype(np.float64))
    z1 = np.exp((s1 - mx).astype(np.float64))
    den = z0 + z1
    w0 = (z0 / den).astype(np.float32)
    w1 = (z1 / den).astype(np.float32)
    return xt, e0, e1, w0, w1


def _fp8(a, scale):
    return np.clip(a * scale, -FP8_MAX, FP8_MAX).astype(ml_dtypes.float8_e4m3)


def _wlay(w, ko, mo):
    """[ko*P, mo*P] -> [P, mo, ko, P]: partition-contiguous DMA layout that
    matches the lhsT access pattern w_sb[:, m, 2k:2k+2, :]."""
    return np.ascontiguousarray(
        np.asarray(w).reshape(ko, P, mo, P).transpose(1, 2, 0, 3)
    )


def kernel(x, gate_w, w1, b1, w2, b2, ln_g, ln_b):
    from concourse.bass_utils import run_bass_kernel_spmd

    x = np.asarray(x)
    xt, e0, e1, wk0, wk1 = _route(x, np.asarray(gate_w))

    # slot assignment: expert e's token list = tokens with e0==e, then e1==e
    idx_e, wv_e = [], []
    for e in range(E):
        i0 = np.nonzero(e0 == e)[0]
        i1 = np.nonzero(e1 == e)[0]
        idx_e.append(np.concatenate([i0, i1]))
        wv_e.append(np.concatenate([wk0[i0], wk1[i1]]))
    maxn = max(len(i) for i in idx_e)
    C = max(CT_MIN, -(-maxn // CT_MIN) * CT_MIN)
    tiles = _tiles_for(C)
    NT = len(tiles)

    use_b2 = bool(np.any(np.asarray(b2) != 0))
    use_lng = bool(np.any(np.asarray(ln_g) != 1))
    use_lnb = bool(np.any(np.asarray(ln_b) != 0))
    key = (C, use_b2, use_lng, use_lnb)
    if key not in _kernel_cache:
        _kernel_cache[key] = _build_bass(C, use_b2, use_lng, use_lnb)
    nc = _kernel_cache[key]

    in_maps = []
    for e in range(E):
        n = len(idx_e[e])
        xTe = np.zeros((D, C), np.float32)
        xTe[:, :n] = xt[idx_e[e]].T
        wve = np.zeros((C,), np.float32)
        wve[:n] = wv_e[e]
        # per-tile padded block layouts: [P, NT, KO1|MO2, CT]
        x8 = _fp8(xTe, SX).reshape(KO1, P, C)
        xr = (xTe * SW).reshape(MO2, P, C)
        xT_blk = np.zeros((P, NT, KO1, CT), ml_dtypes.float8_e4m3)
        xr_blk = np.zeros((P, NT, MO2, CT), np.float32)
        for t, (t0, ct) in enumerate(tiles):
            xT_blk[:, t, :, :ct] = x8[:, :, t0 : t0 + ct].transpose(1, 0, 2)
            xr_blk[:, t, :, :ct] = xr[:, :, t0 : t0 + ct].transpose(1, 0, 2)
        in_maps.append({
            "xT": xT_blk,
            "xTf": xr_blk,  # 2^10-scaled residual (mm2's never-unscaled scale)
            "w1": _fp8(_wlay(w1[e], KO1, MO1), SW),
            "w2": _fp8(_wlay(w2[e], KO2, MO2), SW),
            "b1": np.ascontiguousarray(np.asarray(b1)[e].reshape(MO1, P).T),
            "b2": np.ascontiguousarray((np.asarray(b2)[e] * SW).reshape(MO2, P).T),
            "ln_g": np.ascontiguousarray(np.asarray(ln_g)[e].reshape(MO2, P).T),
            "ln_b": np.ascontiguousarray(np.asarray(ln_b)[e].reshape(MO2, P).T),
            "wv": np.broadcast_to(wve, (P, C)).copy(),
        })

    res = run_bass_kernel_spmd(nc, in_maps, core_ids=list(range(E)))
    kernel.last_results = res

    # combine: token t's two contributions live at known (expert, slot) pairs
    slot0 = np.empty(T, np.int64)
    slot1 = np.empty(T, np.int64)
    for e in range(E):
        n0 = int(np.sum(e0 == e))
        slot0[e0 == e] = np.arange(n0)
        slot1[e1 == e] = n0 + np.arange(int(np.sum(e1 == e)))
    Y = np.stack([res.results[e]["outT"] for e in range(E)])  # [E, D, C]
    Sm = np.stack([res.results[e]["outS"][0] for e in range(E)])  # [E, C]
    out = Y[e0, :, slot0] + Y[e1, :, slot1]  # [T, D]
    # host-side LN mean term: out -= sum_e (wv*rstd*mean)_e ⊗ ln_g_e
    lng = np.asarray(ln_g, np.float32)
    if use_lng:
        out -= Sm[e0, slot0][:, None] * lng[e0] + Sm[e1, slot1][:, None] * lng[e1]
    else:
        out -= (Sm[e0, slot0] + Sm[e1, slot1])[:, None]
    return out.reshape(x.shape).astype(np.float32)


# revision 28
# speedup vs baseline: 1.0463x; 1.0458x over previous
"""MoE (E=8 experts, top-2, D=1024, T=8192) — expert-parallel Trainium2 kernel.

Strategy (per the expert-parallel sharding hint):
  - Host computes the gate (0.1% of FLOPs: scores, top-2, softmax) and uses it
    to shard tokens: each of the 8 NeuronCores owns one expert and receives
    exactly the tokens routed to it (padded to a common capacity C).
  - Each core runs the dense expert FFN + layernorm + combine-weight scaling
    over its routed tokens: 99.9% of the FLOPs.
  - Host gathers the per-expert outputs back into token order (pure gather —
    slot assignment makes a scatter unnecessary) and sums the K=2 contributions.

Device dataflow (activations kept transposed, features on partitions, so the
mm1 -> gelu -> mm2 chain composes with zero transposes):
  xT [D, C] --mm1--> hT [2D, C] --gelu+b1--> mm2 --> + x residual (f32 stream,
  DVE add) --> zT [D, C] --LN scale--> z*(wv*rstd) --> outT

The FFN matmuls run in fp8-e4m3 with perf_mode=DoubleRow (2 MACs/cell/cycle:
the PE virtualizes to 128x256, halving matmul instruction count vs bf16).
Weights are host-prescaled by 2^10 (and x by 2^4 for mm1) so the e4m3 lattice
is used well clear of denormals; the gelu activation unscales mm1 by 2^-14.
mm2's 2^10 output scale is never unscaled: the residual stream arrives as
x*2^10 and layernorm is scale-invariant (eps is scaled by 2^20 to match), so
z' = 2^10*z flows straight through the stats and the final scale.

The LN mean term is applied on the HOST: out = wv*rstd*(z - mean) splits into
wv*rstd*z (device) minus (wv*rstd*mean) (a per-token scalar the device ships
as a 1-row tensor), so the device normalize is one DVE multiply per chunk
instead of subtract+multiply.

LN statistics are computed on the PE: an accumulating ones[128,128].T @ zT
matmul (f32r - fp32 truncated to fp22, full PE rate at >=256 cols) sums over
the feature (partition) axis AND broadcasts across all 128 partitions in one
instruction. w1/w2 are fully SBUF-resident in fp8 (4 MB), host-relaid so every
DMA lands as one fat contiguous descriptor per partition (the [m, ko, col]
nesting matches the lhsT access pattern). mm1 is software-pipelined one tile
ahead of mm2/LN, with mm2 accumulation groups interleaved between mm1 groups
so the in-order PE always has an independent group queued behind any group
waiting on DMA or the DVE.
"""

import sys

sys.path.insert(0, "/opt/trn_rl_repo")

import ml_dtypes
import numpy as np

E, K, D = 8, 2, 1024
H = 2 * D  # ffn hidden
B, S = 2, 4096
T = B * S
CT = 512  # max token tile (PSUM bank = 512 f32)
CT_MIN = 128  # remainder-tile granularity
P = 128
KO1 = D // P  # 8  k-chunks for mm1 (4 DoubleRow pairs)
MO1 = H // P  # 16 m-chunks for mm1
KO2 = H // P  # 16 k-chunks for mm2 (8 DoubleRow pairs)
MO2 = D // P  # 8  m-chunks for mm2

SW = 1024.0  # weight prescale (2^10)
SX = 16.0  # x prescale for the fp8 mm1 moving operand (2^4)
FP8_MAX = 240.0  # TRN float8e4 max normal

_kernel_cache = {}


def _tiles_for(C):
    tiles = [(i * CT, CT) for i in range(C // CT)]
    if C % CT:
        tiles.append((C - C % CT, C % CT))
    return tiles


def _build_bass(C, use_b2, use_lng, use_lnb):
    """Build the per-core expert-FFN kernel for capacity C (multiple of CT_MIN)."""
    import concourse.tile as tile
    from concourse import bacc, mybir

    f32 = mybir.dt.float32
    f32r = mybir.dt.float32r
    fp8 = mybir.dt.float8e4
    AF = mybir.ActivationFunctionType
    OP = mybir.AluOpType
    DR = mybir.MatmulPerfMode.DoubleRow

    assert C % CT_MIN == 0
    tiles = _tiles_for(C)
    NT = len(tiles)

    nc = bacc.Bacc("TRN2", target_bir_lowering=False, debug=False)
    # xT/xres ship in per-tile-padded block layouts so each tile's DMA is one
    # contiguous descriptor per partition
    xT_d = nc.dram_tensor("xT", [P, NT, KO1, CT], fp8, kind="ExternalInput")
    xTf_d = nc.dram_tensor("xTf", [P, NT, MO2, CT], f32r, kind="ExternalInput")
    w1_d = nc.dram_tensor("w1", [P, MO1, KO1, P], fp8, kind="ExternalInput")
    w2_d = nc.dram_tensor("w2", [P, MO2, KO2, P], fp8, kind="ExternalInput")
    # host pre-transposes the per-feature constants to [P, mo] so each is a
    # single fat DMA descriptor instead of ~2k 4-byte gathers
    b1_d = nc.dram_tensor("b1", [P, MO1], f32, kind="ExternalInput")
    b2_d = nc.dram_tensor("b2", [P, MO2], f32, kind="ExternalInput")
    lng_d = nc.dram_tensor("ln_g", [P, MO2], f32, kind="ExternalInput")
    lnb_d = nc.dram_tensor("ln_b", [P, MO2], f32, kind="ExternalInput")
    wv_d = nc.dram_tensor("wv", [P, C], f32, kind="ExternalInput")
    out_d = nc.dram_tensor("outT", [D, C], f32, kind="ExternalOutput")
    outS_d = nc.dram_tensor("outS", [1, C], f32, kind="ExternalOutput")

    out_r = out_d.rearrange("(mo p) c -> p mo c", p=P)

    with tile.TileContext(nc) as tc:
        with (
            tc.tile_pool(name="singles", bufs=1) as singles,
            tc.tile_pool(name="xp", bufs=3) as xp,
            tc.tile_pool(name="hp", bufs=2) as hp,
            tc.tile_pool(name="zp", bufs=3) as zp,
            tc.tile_pool(name="zqp", bufs=6) as zqp,
            tc.tile_pool(name="wvp", bufs=3) as wvp,
            tc.tile_pool(name="xrp", bufs=2) as xrp,
            tc.tile_pool(name="stp", bufs=3) as stp,
            tc.tile_pool(name="ocp", bufs=6) as ocp,
            tc.tile_pool(name="psmm", bufs=6, space="PSUM") as psmm,
            tc.tile_pool(name="psst", bufs=1, space="PSUM") as psst,
        ):
            # ---- resident data ----
            # tile 0's xT first (SP ring is FIFO), then w1 in 8 chunks
            # alternating across both HWDGE rings so tile 0's matmuls never
            # outrun the weight stream; w2 is deferred to mid-tile-0 emission
            # so it does not steal HBM bandwidth from w1
            xT_tiles = [xp.tile([P, KO1, CT], fp8, name="xT_sb") for _ in range(3)]
            nc.sync.dma_start(xT_tiles[0][:], xT_d[:, 0])
            ones_f32 = singles.tile([P, P], f32)
            nc.vector.memset(ones_f32[:], 1.0)
            ones_sb = singles.tile([P, P], f32r)
            nc.vector.tensor_copy(ones_sb[:], ones_f32[:])
            eps_sb = singles.tile([P, 1], f32)
            nc.vector.memset(eps_sb[:], 1e-6 * SW * SW)  # eps on 2^10-scaled z
            # each weight chunk is its own tile: the tile dep-tracker is
            # per-tile, so a single resident tile would stall the first matmul
            # until every chunk DMA lands
            w1_ch = [singles.tile([P, 2, KO1, P], fp8, name=f"w1c{i}") for i in range(MO1 // 2)]
            for mq in range(MO1 // 2):
                eng = nc.sync if mq % 2 == 0 else nc.scalar
                eng.dma_start(w1_ch[mq][:], w1_d[:, 2 * mq : 2 * mq + 2])
            b1_sb = singles.tile([P, MO1], f32)
            nc.scalar.dma_start(b1_sb[:], b1_d[:])
            b2_sb = singles.tile([P, MO2], f32)
            nc.scalar.dma_start(b2_sb[:], b2_d[:])
            lng_sb = singles.tile([P, MO2], f32)
            nc.scalar.dma_start(lng_sb[:], lng_d[:])
            lnb_sb = singles.tile([P, MO2], f32)
            nc.scalar.dma_start(lnb_sb[:], lnb_d[:])
            w2_ch = [singles.tile([P, 2, KO2, P], fp8, name=f"w2c{i}") for i in range(MO2 // 2)]

            def emit_w2_chunk(mq):
                nc.scalar.dma_start(w2_ch[mq][:], w2_d[:, 2 * mq : 2 * mq + 2])

            def emit_mm1(t, t0, ct, xT_sb, st, prev_st=None):
                """mm1 (fp8 DoubleRow) + gelu into a fresh hT tile; mm2 groups
                of the previous tile interleave so the PE always has an
                independent accumulation group queued behind a stalled one."""
                hT_sb = hp.tile([P, KO2, CT], fp8, name="hT_sb")
                for m in range(MO1):
                    ps = psmm.tile([P, CT], f32, name="ps_mm")
                    for k in range(KO1 // 2):
                        nc.tensor.matmul(
                            ps[:, :ct],
                            w1_ch[m // 2][:, m % 2, 2 * k : 2 * k + 2, :],
                            xT_sb[:, 2 * k : 2 * k + 2, :ct],
                            start=(k == 0),
                            stop=(k == KO1 // 2 - 1),
                            perf_mode=DR,
                        )
                    # gelu(2^-14 * ps + b1) -> fp8 h (unscaled)
                    nc.scalar.activation(
                        out=hT_sb[:, m, :ct],
                        in_=ps[:, :ct],
                        func=AF.Gelu,
                        bias=b1_sb[:, m : m + 1],
                        scale=1.0 / (SW * SX),
                    )
                    if t == 0 and m in (4, 6, 8, 10):
                        emit_w2_chunk((m - 4) // 2)
                    if m == (6 if t == 0 else 2):
                        nc.scalar.dma_start(st["wv_sb"], wv_d[:, st["ts"]])
                        nc.scalar.dma_start(st["xres"][:, 0:4], xTf_d[:, t, 0:4])
                        nc.scalar.dma_start(st["xres"][:, 4:8], xTf_d[:, t, 4:8])
                    if prev_st is not None and m % 2 == 1:
                        mm2_group(prev_st, m // 2)
                st["hT_sb"] = hT_sb
                return hT_sb

            def mm2_state(t, t0, ct):
                """Allocated ahead of mm1(t); the xres/wv DMAs are emitted
                inside emit_mm1(t) behind a few of tile t's gelus, so they
                start a full tile before mm2(t) consumes them but never steal
                HBM bandwidth from the startup weight stream."""
                ts = slice(t0, t0 + ct)
                wv_sb = wvp.tile([P, CT], f32, name="wv_sb")[:, :ct]
                xres = xrp.tile([P, MO2, CT], f32r, name="xres")
                return {
                    "t": t, "t0": t0, "ct": ct, "ts": ts, "hT_sb": None,
                    "wv_sb": wv_sb, "xres": xres, "norm_prev": None,
                    "zT_sb": zp.tile([P, MO2, CT], f32r, name="zT_sb"),
                    "ps_S": psst.tile([P, CT], f32, name="ps_S"),
                    "ps_Q": psst.tile([P, CT], f32, name="ps_Q"),
                    "pending": [],
                }

            def mm2_stats(st, mo, zc, zq):
                ct = st["ct"]
                nc.tensor.matmul(
                    st["ps_S"][:, :ct], ones_sb[:], zc,
                    start=(mo == 0), stop=(mo == MO2 - 1), skip_group_check=True,
                )
                nc.tensor.matmul(
                    st["ps_Q"][:, :ct], ones_sb[:], zq,
                    start=(mo == 0), stop=(mo == MO2 - 1), skip_group_check=True,
                )

            def mm2_group(st, mo):
                ct = st["ct"]
                ps = psmm.tile([P, CT], f32, name="ps_mm")
                for k in range(KO2 // 2):
                    nc.tensor.matmul(
                        ps[:, :ct],
                        w2_ch[mo // 2][:, mo % 2, 2 * k : 2 * k + 2, :],
                        st["hT_sb"][:, 2 * k : 2 * k + 2, :ct],
                        start=(k == 0),
                        stop=(k == KO2 // 2 - 1),
                        perf_mode=DR,
                    )
                if st["pending"]:
                    mm2_stats(st, *st["pending"].pop())
                zc = st["zT_sb"][:, mo, :ct]
                nc.vector.tensor_tensor(zc, ps[:, :ct], st["xres"][:, mo, :ct], OP.add)
                if use_b2:
                    nc.vector.tensor_scalar_add(zc, zc, b2_sb[:, mo : mo + 1])
                zq = zqp.tile([P, CT], f32r, name="zq")[:, :ct]
                nc.vector.tensor_tensor(zq, zc, zc, OP.mult)
                st["pending"].append((mo, zc, zq))
                # the previous tile's normalize rides one chunk at a time
                # behind this tile's z chain, so the in-order DVE never delays
                # the stats matmuls by a whole tile's worth of normalize work
                if st["norm_prev"] is not None:
                    ln_norm_chunk(st["norm_prev"], mo)

            def ln_stats(st):
                ct, ts, wv_sb = st["ct"], st["ts"], st["wv_sb"]
                mm2_stats(st, *st["pending"].pop())
                # LN stats: mean/rstd replicated across partitions
                mean_sb = stp.tile([P, CT], f32, name="mean")[:, :ct]
                nc.vector.tensor_scalar_mul(mean_sb, st["ps_S"][:, :ct], 1.0 / D)
                rw_sb = stp.tile([P, CT], f32, name="rw")[:, :ct]
                nc.vector.tensor_scalar_mul(rw_sb, st["ps_Q"][:, :ct], 1.0 / D)
                msq_sb = zqp.tile([P, CT], f32, name="zq")[:, :ct]
                nc.vector.tensor_tensor(msq_sb, mean_sb, mean_sb, OP.mult)
                nc.vector.tensor_tensor(rw_sb, rw_sb, msq_sb, OP.subtract)
                # rw = wv / sqrt(var + eps): sqrt on ACT, fast-reciprocal on DVE
                nc.scalar.activation(
                    out=rw_sb, in_=rw_sb, func=AF.Sqrt, bias=eps_sb[:], scale=1.0,
                )
                nc.vector.reciprocal_approx_fast(rw_sb, rw_sb)
                nc.vector.tensor_tensor(rw_sb, rw_sb, wv_sb, OP.mult)
                # per-token mean term, applied host-side after the combine
                mrw_sb = zqp.tile([P, CT], f32, name="zq")[:, :ct]
                nc.vector.tensor_tensor(mrw_sb, mean_sb, rw_sb, OP.mult)
                nc.scalar.dma_start(outS_d[0:1, ts], mrw_sb[0:1, :ct])
                st["rw_sb"] = rw_sb

            def ln_norm_chunk(st, mo):
                ct = st["ct"]
                occ = ocp.tile([P, CT], f32, name="oc")[:, :ct]
                nc.vector.tensor_tensor(occ, st["zT_sb"][:, mo, :ct], st["rw_sb"], OP.mult)
                if use_lng:
                    nc.vector.tensor_scalar_mul(occ, occ, lng_sb[:, mo : mo + 1])
                if use_lnb:
                    lb = stp.tile([P, CT], f32, name="lb")[:, :ct]
                    nc.vector.tensor_scalar_mul(lb, st["wv_sb"], lnb_sb[:, mo : mo + 1])
                    nc.vector.tensor_tensor(occ, occ, lb, OP.add)
                nc.sync.dma_start(out_r[:, mo, st["ts"]], occ)

            # xT(t) is consumed only by mm1(t) (residual uses the xres
            # stream); prefetches are queued after the w1 chunks so they never
            # delay the critical w1 stream
            prev_st = None
            for t, (t0, ct) in enumerate(tiles):
                st = mm2_state(t, t0, ct)
                emit_mm1(t, t0, ct, xT_tiles[t % 3], st, prev_st)
                for tn_i in ([1, 2] if t == 0 else [t + 2]):
                    if tn_i < NT:
                        nc.sync.dma_start(xT_tiles[tn_i % 3][:], xT_d[:, tn_i])
                if prev_st is not None:
                    ln_stats(prev_st)
                    st["norm_prev"] = prev_st
                prev_st = st
            for mo in range(MO2):
                mm2_group(prev_st, mo)
            ln_stats(prev_st)
            for mo in range(MO2):
                ln_norm_chunk(prev_st, mo)

    nc.finalize()
    return nc


def _route(x, gate_w):
    """Host gate: top-2 per token + softmax combine weights (matches
    jax.lax.top_k tie-breaking: lower index wins)."""
    xt = x.reshape(-1, D)
    scores = xt.astype(np.float32) @ gate_w.astype(np.float32)  # [T, E]
    e0 = np.argmax(scores, axis=1)
    s0 = scores[np.arange(T), e0]
    masked = scores.copy()
    masked[np.arange(T), e0] = -np.inf
    e1 = np.argmax(masked, axis=1)
    s1 = masked[np.arange(T), e1]
    # softmax over the two selected scores
    mx = np.maximum(s0, s1)
    z0 = np.exp((s0 - mx).astype(np.float64))
    z1 = np.exp((s1 - mx).astype(np.float64))
    den = z0 + z1
    w0 = (z0 / den).astype(np.float32)
    w1 = (z1 / den).astype(np.float32)
    return xt, e0, e1, w0, w1


def _fp8(a, scale):
    return np.clip(a * scale, -FP8_MAX, FP8_MAX).astype(ml_dtypes.float8_e4m3)


def _wlay(w, ko, mo):
    """[ko*P, mo*P] -> [P, mo, ko, P]: partition-contiguous DMA layout that
    matches the lhsT access pattern w_sb[:, m, 2k:2k+2, :]."""
    return np.ascontiguousarray(
        np.asarray(w).reshape(ko, P, mo, P).transpose(1, 2, 0, 3)
    )


def kernel(x, gate_w, w1, b1, w2, b2, ln_g, ln_b):
    from concourse.bass_utils import run_bass_kernel_spmd

    x = np.asarray(x)
    xt, e0, e1, wk0, wk1 = _route(x, np.asarray(gate_w))

    # slot assignment: expert e's token list = tokens with e0==e, then e1==e
    idx_e, wv_e = [], []
    for e in range(E):
        i0 = np.nonzero(e0 == e)[0]
        i1 = np.nonzero(e1 == e)[0]
        idx_e.append(np.concatenate([i0, i1]))
        wv_e.append(np.concatenate([wk0[i0], wk1[i1]]))
    maxn = max(len(i) for i in idx_e)
    C = max(CT_MIN, -(-maxn // CT_MIN) * CT_MIN)
    tiles = _tiles_for(C)
    NT = len(tiles)

    use_b2 = bool(np.any(np.asarray(b2) != 0))
    use_lng = bool(np.any(np.asarray(ln_g) != 1))
    use_lnb = bool(np.any(np.asarray(ln_b) != 0))
    key = (C, use_b2, use_lng, use_lnb)
    if key not in _kernel_cache:
        _kernel_cache[key] = _build_bass(C, use_b2, use_lng, use_lnb)
    nc = _kernel_cache[key]

    in_maps = []
    for e in range(E):
        n = len(idx_e[e])
        xTe = np.zeros((D, C), np.float32)
        xTe[:, :n] = xt[idx_e[e]].T
        wve = np.zeros((C,), np.float32)
        wve[:n] = wv_e[e]
        # per-tile padded block layouts: [P, NT, KO1|MO2, CT]
        x8 = _fp8(xTe, SX).reshape(KO1, P, C)
        xr = (xTe * SW).reshape(MO2, P, C)
        xT_blk = np.zeros((P, NT, KO1, CT), ml_dtypes.float8_e4m3)
        xr_blk = np.zeros((P, NT, MO2, CT), np.float32)
        for t, (t0, ct) in enumerate(tiles):
            xT_blk[:, t, :, :ct] = x8[:, :, t0 : t0 + ct].transpose(1, 0, 2)
            xr_blk[:, t, :, :ct] = xr[:, :, t0 : t0 + ct].transpose(1, 0, 2)
        in_maps.append({
            "xT": xT_blk,
            "xTf": xr_blk,  # 2^10-scaled residual (mm2's never-unscaled scale)
            "w1": _fp8(_wlay(w1[e], KO1, MO1), SW),
            "w2": _fp8(_wlay(w2[e], KO2, MO2), SW),
            "b1": np.ascontiguousarray(np.asarray(b1)[e].reshape(MO1, P).T),
            "b2": np.ascontiguousarray((np.asarray(b2)[e] * SW).reshape(MO2, P).T),
            "ln_g": np.ascontiguousarray(np.asarray(ln_g)[e].reshape(MO2, P).T),
            "ln_b": np.ascontiguousarray(np.asarray(ln_b)[e].reshape(MO2, P).T),
            "wv": np.broadcast_to(wve, (P, C)).copy(),
        })

    res = run_bass_kernel_spmd(nc, in_maps, core_ids=list(range(E)))
    kernel.last_results = res

    # combine: token t's two contributions live at known (expert, slot) pairs
    slot0 = np.empty(T, np.int64)
    slot1 = np.empty(T, np.int64)
    for e in range(E):
        n0 = int(np.sum(e0 == e))
        slot0[e0 == e] = np.arange(n0)
        slot1[e1 == e] = n0 + np.arange(int(np.sum(e1 == e)))
    Y = np.stack([res.results[e]["outT"] for e in range(E)])  # [E, D, C]
    Sm = np.stack([res.results[e]["outS"][0] for e in range(E)])  # [E, C]
    out = Y[e0, :, slot0] + Y[e1, :, slot1]  # [T, D]
    # host-side LN mean term: out -= sum_e (wv*rstd*mean)_e ⊗ ln_g_e
    lng = np.asarray(ln_g, np.float32)
    if use_lng:
        out -= Sm[e0, slot0][:, None] * lng[e0] + Sm[e1, slot1][:, None] * lng[e1]
    else:
        out -= (Sm[e0, slot0] + Sm[e1, slot1])[:, None]
    return out.reshape(x.shape).astype(np.float32)


# revision 29
# speedup vs baseline: 1.1035x; 1.0547x over previous
"""MoE (E=8 experts, top-2, D=1024, T=8192) — expert-parallel Trainium2 kernel.

Strategy (per the expert-parallel sharding hint):
  - Host computes the gate (0.1% of FLOPs: scores, top-2, softmax) and uses it
    to shard tokens: each of the 8 NeuronCores owns one expert and receives
    exactly the tokens routed to it (padded to a common capacity C).
  - Each core runs the dense expert FFN + layernorm + combine-weight scaling
    over its routed tokens: 99.9% of the FLOPs.
  - Host gathers the per-expert outputs back into token order (pure gather —
    slot assignment makes a scatter unnecessary) and sums the K=2 contributions.

Device dataflow (activations kept transposed, features on partitions, so the
mm1 -> gelu -> mm2 chain composes with zero transposes):
  xT [D, C] --mm1--> hT [2D, C] --gelu+b1--> mm2 --> + x residual (f32 stream,
  DVE add) --> zT [D, C] --LN scale--> z*(wv*rstd) --> outT

The FFN matmuls run in fp8-e4m3 with perf_mode=DoubleRow (2 MACs/cell/cycle:
the PE virtualizes to 128x256, halving matmul instruction count vs bf16).
Weights are host-prescaled by 2^10 (and x by 2^4 for mm1) so the e4m3 lattice
is used well clear of denormals; the gelu activation unscales mm1 by 2^-14.
mm2's 2^10 output scale is never unscaled: the residual stream arrives as
x*2^10 and layernorm is scale-invariant (eps is scaled by 2^20 to match), so
z' = 2^10*z flows straight through the stats and the final scale.

The LN mean term is applied on the HOST: out = wv*rstd*(z - mean) splits into
wv*rstd*z (device) minus (wv*rstd*mean) (a per-token scalar the device ships
as a 1-row tensor), so the device normalize is one DVE multiply per chunk
instead of subtract+multiply.

LN statistics are computed on the PE: an accumulating ones[128,128].T @ zT
matmul (f32r - fp32 truncated to fp22, full PE rate at >=256 cols) sums over
the feature (partition) axis AND broadcasts across all 128 partitions in one
instruction. w1/w2 are fully SBUF-resident in fp8 (4 MB), host-relaid so every
DMA lands as one fat contiguous descriptor per partition (the [m, ko, col]
nesting matches the lhsT access pattern). mm1 is software-pipelined one tile
ahead of mm2/LN, with mm2 accumulation groups interleaved between mm1 groups
so the in-order PE always has an independent group queued behind any group
waiting on DMA or the DVE.
"""

import sys

sys.path.insert(0, "/opt/trn_rl_repo")

import ml_dtypes
import numpy as np

E, K, D = 8, 2, 1024
H = 2 * D  # ffn hidden
B, S = 2, 4096
T = B * S
CT = 512  # max token tile (PSUM bank = 512 f32)
CT_MIN = 128  # remainder-tile granularity
P = 128
KO1 = D // P  # 8  k-chunks for mm1 (4 DoubleRow pairs)
MO1 = H // P  # 16 m-chunks for mm1
KO2 = H // P  # 16 k-chunks for mm2 (8 DoubleRow pairs)
MO2 = D // P  # 8  m-chunks for mm2

SW = 1024.0  # weight prescale (2^10)
SX = 16.0  # x prescale for the fp8 mm1 moving operand (2^4)
FP8_MAX = 240.0  # TRN float8e4 max normal

_kernel_cache = {}


def _tiles_for(C):
    tiles = [(i * CT, CT) for i in range(C // CT)]
    if C % CT:
        tiles.append((C - C % CT, C % CT))
    return tiles


def _build_bass(C, use_b2, use_lng, use_lnb):
    """Build the per-core expert-FFN kernel for capacity C (multiple of CT_MIN)."""
    import concourse.tile as tile
    from concourse import bacc, mybir

    f32 = mybir.dt.float32
    f32r = mybir.dt.float32r
    fp8 = mybir.dt.float8e4
    AF = mybir.ActivationFunctionType
    OP = mybir.AluOpType
    DR = mybir.MatmulPerfMode.DoubleRow

    assert C % CT_MIN == 0
    tiles = _tiles_for(C)
    NT = len(tiles)

    nc = bacc.Bacc("TRN2", target_bir_lowering=False, debug=False)
    # xT/xres ship in per-tile-padded block layouts so each tile's DMA is one
    # contiguous descriptor per partition
    xT_d = nc.dram_tensor("xT", [P, NT, KO1, CT], fp8, kind="ExternalInput")
    xTf_d = nc.dram_tensor("xTf", [P, NT, MO2, CT], f32r, kind="ExternalInput")
    w1_d = nc.dram_tensor("w1", [P, MO1, KO1, P], fp8, kind="ExternalInput")
    w2_d = nc.dram_tensor("w2", [P, MO2, KO2, P], fp8, kind="ExternalInput")
    # host pre-transposes the per-feature constants to [P, mo] so each is a
    # single fat DMA descriptor instead of ~2k 4-byte gathers
    b1_d = nc.dram_tensor("b1", [P, MO1], f32, kind="ExternalInput")
    b2_d = nc.dram_tensor("b2", [P, MO2], f32, kind="ExternalInput")
    lng_d = nc.dram_tensor("ln_g", [P, MO2], f32, kind="ExternalInput")
    lnb_d = nc.dram_tensor("ln_b", [P, MO2], f32, kind="ExternalInput")
    wv_d = nc.dram_tensor("wv", [P, C], f32, kind="ExternalInput")
    out_d = nc.dram_tensor("outT", [D, C], f32, kind="ExternalOutput")
    outS_d = nc.dram_tensor("outS", [1, C], f32, kind="ExternalOutput")

    out_r = out_d.rearrange("(mo p) c -> p mo c", p=P)

    with tile.TileContext(nc) as tc:
        with (
            tc.tile_pool(name="singles", bufs=1) as singles,
            tc.tile_pool(name="xp", bufs=3) as xp,
            tc.tile_pool(name="hp", bufs=2) as hp,
            tc.tile_pool(name="zp", bufs=3) as zp,
            tc.tile_pool(name="zqp", bufs=6) as zqp,
            tc.tile_pool(name="wvp", bufs=3) as wvp,
            tc.tile_pool(name="xrp", bufs=2) as xrp,
            tc.tile_pool(name="stp", bufs=3) as stp,
            tc.tile_pool(name="ocp", bufs=6) as ocp,
            tc.tile_pool(name="psmm", bufs=6, space="PSUM") as psmm,
            tc.tile_pool(name="psst", bufs=1, space="PSUM") as psst,
        ):
            # ---- resident data ----
            # tile 0's xT first (SP ring is FIFO), then w1 in 8 chunks
            # alternating across both HWDGE rings so tile 0's matmuls never
            # outrun the weight stream; w2 is deferred to mid-tile-0 emission
            # so it does not steal HBM bandwidth from w1
            xT_tiles = [xp.tile([P, KO1, CT], fp8, name="xT_sb") for _ in range(3)]
            nc.sync.dma_start(xT_tiles[0][:], xT_d[:, 0])
            ones_f32 = singles.tile([P, P], f32)
            nc.vector.memset(ones_f32[:], 1.0)
            ones_sb = singles.tile([P, P], f32r)
            nc.vector.tensor_copy(ones_sb[:], ones_f32[:])
            eps_sb = singles.tile([P, 1], f32)
            nc.vector.memset(eps_sb[:], 1e-6 * SW * SW)  # eps on 2^10-scaled z
            # each weight chunk is its own tile: the tile dep-tracker is
            # per-tile, so a single resident tile would stall the first matmul
            # until every chunk DMA lands
            w1_ch = [singles.tile([P, 2, KO1, P], fp8, name=f"w1c{i}") for i in range(MO1 // 2)]
            for mq in range(MO1 // 2):
                eng = nc.sync if mq % 2 == 0 else nc.scalar
                eng.dma_start(w1_ch[mq][:], w1_d[:, 2 * mq : 2 * mq + 2])
            b1_sb = singles.tile([P, MO1], f32)
            nc.scalar.dma_start(b1_sb[:], b1_d[:])
            b2_sb = singles.tile([P, MO2], f32)
            nc.scalar.dma_start(b2_sb[:], b2_d[:])
            lng_sb = singles.tile([P, MO2], f32)
            nc.scalar.dma_start(lng_sb[:], lng_d[:])
            lnb_sb = singles.tile([P, MO2], f32)
            nc.scalar.dma_start(lnb_sb[:], lnb_d[:])
            w2_ch = [singles.tile([P, 2, KO2, P], fp8, name=f"w2c{i}") for i in range(MO2 // 2)]

            def emit_w2_chunk(mq):
                nc.scalar.dma_start(w2_ch[mq][:], w2_d[:, 2 * mq : 2 * mq + 2])

            def emit_mm1(t, t0, ct, xT_sb, st, prev_st=None):
                """mm1 (fp8 DoubleRow) + gelu into a fresh hT tile; mm2 groups
                of the previous tile interleave so the PE always has an
                independent accumulation group queued behind a stalled one."""
                hT_sb = hp.tile([P, KO2, CT], fp8, name="hT_sb")
                for m in range(MO1):
                    ps = psmm.tile([P, CT], f32, name="ps_mm")
                    for k in range(KO1 // 2):
                        nc.tensor.matmul(
                            ps[:, :ct],
                            w1_ch[m // 2][:, m % 2, 2 * k : 2 * k + 2, :],
                            xT_sb[:, 2 * k : 2 * k + 2, :ct],
                            start=(k == 0),
                            stop=(k == KO1 // 2 - 1),
                            perf_mode=DR,
                        )
                    # gelu(2^-14 * ps + b1) -> fp8 h (unscaled)
                    nc.scalar.activation(
                        out=hT_sb[:, m, :ct],
                        in_=ps[:, :ct],
                        func=AF.Gelu,
                        bias=b1_sb[:, m : m + 1],
                        scale=1.0 / (SW * SX),
                    )
                    if t == 0 and m in (4, 6, 8, 10):
                        emit_w2_chunk((m - 4) // 2)
                    if m == (6 if t == 0 else 2):
                        nc.scalar.dma_start(st["wv_sb"], wv_d[:, st["ts"]])
                        nc.scalar.dma_start(st["xres"][:, 0:4], xTf_d[:, t, 0:4])
                        nc.scalar.dma_start(st["xres"][:, 4:8], xTf_d[:, t, 4:8])
                    if prev_st is not None and 1 <= m <= MO2:
                        mm2_group(prev_st, m - 1)
                st["hT_sb"] = hT_sb
                return hT_sb

            def mm2_state(t, t0, ct):
                """Allocated ahead of mm1(t); the xres/wv DMAs are emitted
                inside emit_mm1(t) behind a few of tile t's gelus, so they
                start a full tile before mm2(t) consumes them but never steal
                HBM bandwidth from the startup weight stream."""
                ts = slice(t0, t0 + ct)
                wv_sb = wvp.tile([P, CT], f32, name="wv_sb")[:, :ct]
                xres = xrp.tile([P, MO2, CT], f32r, name="xres")
                return {
                    "t": t, "t0": t0, "ct": ct, "ts": ts, "hT_sb": None,
                    "wv_sb": wv_sb, "xres": xres, "norm_prev": None,
                    "zT_sb": zp.tile([P, MO2, CT], f32r, name="zT_sb"),
                    "ps_S": psst.tile([P, CT], f32, name="ps_S"),
                    "ps_Q": psst.tile([P, CT], f32, name="ps_Q"),
                    "pending": [],
                }

            def mm2_stats(st, mo, zc, zq):
                ct = st["ct"]
                nc.tensor.matmul(
                    st["ps_S"][:, :ct], ones_sb[:], zc,
                    start=(mo == 0), stop=(mo == MO2 - 1), skip_group_check=True,
                )
                nc.tensor.matmul(
                    st["ps_Q"][:, :ct], ones_sb[:], zq,
                    start=(mo == 0), stop=(mo == MO2 - 1), skip_group_check=True,
                )

            def mm2_group(st, mo):
                ct = st["ct"]
                ps = psmm.tile([P, CT], f32, name="ps_mm")
                for k in range(KO2 // 2):
                    nc.tensor.matmul(
                        ps[:, :ct],
                        w2_ch[mo // 2][:, mo % 2, 2 * k : 2 * k + 2, :],
                        st["hT_sb"][:, 2 * k : 2 * k + 2, :ct],
                        start=(k == 0),
                        stop=(k == KO2 // 2 - 1),
                        perf_mode=DR,
                    )
                if st["pending"]:
                    mm2_stats(st, *st["pending"].pop())
                zc = st["zT_sb"][:, mo, :ct]
                nc.vector.tensor_tensor(zc, ps[:, :ct], st["xres"][:, mo, :ct], OP.add)
                if use_b2:
                    nc.vector.tensor_scalar_add(zc, zc, b2_sb[:, mo : mo + 1])
                zq = zqp.tile([P, CT], f32r, name="zq")[:, :ct]
                nc.vector.tensor_tensor(zq, zc, zc, OP.mult)
                st["pending"].append((mo, zc, zq))
                # the previous tile's normalize rides one chunk at a time
                # behind this tile's z chain, so the in-order DVE never delays
                # the stats matmuls by a whole tile's worth of normalize work
                if st["norm_prev"] is not None:
                    ln_norm_chunk(st["norm_prev"], mo)

            def ln_stats(st):
                ct, ts, wv_sb = st["ct"], st["ts"], st["wv_sb"]
                mm2_stats(st, *st["pending"].pop())
                # LN stats: mean/rstd replicated across partitions
                mean_sb = stp.tile([P, CT], f32, name="mean")[:, :ct]
                nc.vector.tensor_scalar_mul(mean_sb, st["ps_S"][:, :ct], 1.0 / D)
                rw_sb = stp.tile([P, CT], f32, name="rw")[:, :ct]
                nc.vector.tensor_scalar_mul(rw_sb, st["ps_Q"][:, :ct], 1.0 / D)
                msq_sb = zqp.tile([P, CT], f32, name="zq")[:, :ct]
                nc.vector.tensor_tensor(msq_sb, mean_sb, mean_sb, OP.mult)
                nc.vector.tensor_tensor(rw_sb, rw_sb, msq_sb, OP.subtract)
                # rw = wv / sqrt(var + eps): sqrt on ACT, fast-reciprocal on DVE
                nc.scalar.activation(
                    out=rw_sb, in_=rw_sb, func=AF.Sqrt, bias=eps_sb[:], scale=1.0,
                )
                nc.vector.reciprocal_approx_fast(rw_sb, rw_sb)
                nc.vector.tensor_tensor(rw_sb, rw_sb, wv_sb, OP.mult)
                # per-token mean term, applied host-side after the combine
                mrw_sb = zqp.tile([P, CT], f32, name="zq")[:, :ct]
                nc.vector.tensor_tensor(mrw_sb, mean_sb, rw_sb, OP.mult)
                nc.scalar.dma_start(outS_d[0:1, ts], mrw_sb[0:1, :ct])
                st["rw_sb"] = rw_sb

            def ln_norm_chunk(st, mo):
                ct = st["ct"]
                occ = ocp.tile([P, CT], f32, name="oc")[:, :ct]
                nc.vector.tensor_tensor(occ, st["zT_sb"][:, mo, :ct], st["rw_sb"], OP.mult)
                if use_lng:
                    nc.vector.tensor_scalar_mul(occ, occ, lng_sb[:, mo : mo + 1])
                if use_lnb:
                    lb = stp.tile([P, CT], f32, name="lb")[:, :ct]
                    nc.vector.tensor_scalar_mul(lb, st["wv_sb"], lnb_sb[:, mo : mo + 1])
                    nc.vector.tensor_tensor(occ, occ, lb, OP.add)
                nc.sync.dma_start(out_r[:, mo, st["ts"]], occ)

            # xT(t) is consumed only by mm1(t) (residual uses the xres
            # stream); prefetches are queued after the w1 chunks so they never
            # delay the critical w1 stream
            prev_st = None
            for t, (t0, ct) in enumerate(tiles):
                st = mm2_state(t, t0, ct)
                emit_mm1(t, t0, ct, xT_tiles[t % 3], st, prev_st)
                for tn_i in ([1, 2] if t == 0 else [t + 2]):
                    if tn_i < NT:
                        nc.sync.dma_start(xT_tiles[tn_i % 3][:], xT_d[:, tn_i])
                if prev_st is not None:
                    ln_stats(prev_st)
                    st["norm_prev"] = prev_st
                prev_st = st
            for mo in range(MO2):
                mm2_group(prev_st, mo)
            ln_stats(prev_st)
            for mo in range(MO2):
                ln_norm_chunk(prev_st, mo)

    nc.finalize()
    return nc


def _route(x, gate_w):
    """Host gate: top-2 per token + softmax combine weights (matches
    jax.lax.top_k tie-breaking: lower index wins)."""
    xt = x.reshape(-1, D)
    scores = xt.astype(np.float32) @ gate_w.astype(np.float32)  # [T, E]
    e0 = np.argmax(scores, axis=1)
    s0 = scores[np.arange(T), e0]
    masked = scores.copy()
    masked[np.arange(T), e0] = -np.inf
    e1 = np.argmax(masked, axis=1)
    s1 = masked[np.arange(T), e1]
    # softmax over the two selected scores
    mx = np.maximum(s0, s1)
    z0 = np.exp((s0 - mx).astype(np.float64))
    z1 = np.exp((s1 - mx).astype(np.float64))
    den = z0 + z1
    w0 = (z0 / den).astype(np.float32)
    w1 = (z1 / den).astype(np.float32)
    return xt, e0, e1, w0, w1


def _fp8(a, scale):
    return np.clip(a * scale, -FP8_MAX, FP8_MAX).astype(ml_dtypes.float8_e4m3)


def _wlay(w, ko, mo):
    """[ko*P, mo*P] -> [P, mo, ko, P]: partition-contiguous DMA layout that
    matches the lhsT access pattern w_sb[:, m, 2k:2k+2, :]."""
    return np.ascontiguousarray(
        np.asarray(w).reshape(ko, P, mo, P).transpose(1, 2, 0, 3)
    )


def kernel(x, gate_w, w1, b1, w2, b2, ln_g, ln_b):
    from concourse.bass_utils import run_bass_kernel_spmd

    x = np.asarray(x)
    xt, e0, e1, wk0, wk1 = _route(x, np.asarray(gate_w))

    # slot assignment: expert e's token list = tokens with e0==e, then e1==e
    idx_e, wv_e = [], []
    for e in range(E):
        i0 = np.nonzero(e0 == e)[0]
        i1 = np.nonzero(e1 == e)[0]
        idx_e.append(np.concatenate([i0, i1]))
        wv_e.append(np.concatenate([wk0[i0], wk1[i1]]))
    maxn = max(len(i) for i in idx_e)
    C = max(CT_MIN, -(-maxn // CT_MIN) * CT_MIN)
    tiles = _tiles_for(C)
    NT = len(tiles)

    use_b2 = bool(np.any(np.asarray(b2) != 0))
    use_lng = bool(np.any(np.asarray(ln_g) != 1))
    use_lnb = bool(np.any(np.asarray(ln_b) != 0))
    key = (C, use_b2, use_lng, use_lnb)
    if key not in _kernel_cache:
        _kernel_cache[key] = _build_bass(C, use_b2, use_lng, use_lnb)
    nc = _kernel_cache[key]

    in_maps = []
    for e in range(E):
        n = len(idx_e[e])
        xTe = np.zeros((D, C), np.float32)
        xTe[:, :n] = xt[idx_e[e]].T
        wve = np.zeros((C,), np.float32)
        wve[:n] = wv_e[e]
        # per-tile padded block layouts: [P, NT, KO1|MO2, CT]
        x8 = _fp8(xTe, SX).reshape(KO1, P, C)
        xr = (xTe * SW).reshape(MO2, P, C)
        xT_blk = np.zeros((P, NT, KO1, CT), ml_dtypes.float8_e4m3)
        xr_blk = np.zeros((P, NT, MO2, CT), np.float32)
        for t, (t0, ct) in enumerate(tiles):
            xT_blk[:, t, :, :ct] = x8[:, :, t0 : t0 + ct].transpose(1, 0, 2)
            xr_blk[:, t, :, :ct] = xr[:, :, t0 : t0 + ct].transpose(1, 0, 2)
        in_maps.append({
            "xT": xT_blk,
            "xTf": xr_blk,  # 2^10-scaled residual (mm2's never-unscaled scale)
            "w1": _fp8(_wlay(w1[e], KO1, MO1), SW),
            "w2": _fp8(_wlay(w2[e], KO2, MO2), SW),
            "b1": np.ascontiguousarray(np.asarray(b1)[e].reshape(MO1, P).T),
            "b2": np.ascontiguousarray((np.asarray(b2)[e] * SW).reshape(MO2, P).T),
            "ln_g": np.ascontiguousarray(np.asarray(ln_g)[e].reshape(MO2, P).T),
            "ln_b": np.ascontiguousarray(np.asarray(ln_b)[e].reshape(MO2, P).T),
            "wv": np.broadcast_to(wve, (P, C)).copy(),
        })

    res = run_bass_kernel_spmd(nc, in_maps, core_ids=list(range(E)))
    kernel.last_results = res

    # combine: token t's two contributions live at known (expert, slot) pairs
    slot0 = np.empty(T, np.int64)
    slot1 = np.empty(T, np.int64)
    for e in range(E):
        n0 = int(np.sum(e0 == e))
        slot0[e0 == e] = np.arange(n0)
        slot1[e1 == e] = n0 + np.arange(int(np.sum(e1 == e)))
    Y = np.stack([res.results[e]["outT"] for e in range(E)])  # [E, D, C]
    Sm = np.stack([res.results[e]["outS"][0] for e in range(E)])  # [E, C]
    out = Y[e0, :, slot0] + Y[e1, :, slot1]  # [T, D]
    # host-side LN mean term: out -= sum_e (wv*rstd*mean)_e ⊗ ln_g_e
    lng = np.asarray(ln_g, np.float32)
    if use_lng:
        out -= Sm[e0, slot0][:, None] * lng[e0] + Sm[e1, slot1][:, None] * lng[e1]
    else:
        out -= (Sm[e0, slot0] + Sm[e1, slot1])[:, None]
    return out.reshape(x.shape).astype(np.float32)


# revision 30
# speedup vs baseline: 1.1108x; 1.0066x over previous
"""MoE (E=8 experts, top-2, D=1024, T=8192) — expert-parallel Trainium2 kernel.

Strategy (per the expert-parallel sharding hint):
  - Host computes the gate (0.1% of FLOPs: scores, top-2, softmax) and uses it
    to shard tokens: each of the 8 NeuronCores owns one expert and receives
    exactly the tokens routed to it (padded to a common capacity C).
  - Each core runs the dense expert FFN + layernorm + combine-weight scaling
    over its routed tokens: 99.9% of the FLOPs.
  - Host gathers the per-expert outputs back into token order (pure gather —
    slot assignment makes a scatter unnecessary) and sums the K=2 contributions.

Device dataflow (activations kept transposed, features on partitions, so the
mm1 -> gelu -> mm2 chain composes with zero transposes):
  xT [D, C] --mm1--> hT [2D, C] --gelu+b1--> mm2 --> + x residual (f32 stream,
  DVE add) --> zT [D, C] --LN scale--> z*(wv*rstd) --> outT

The FFN matmuls run in fp8-e4m3 with perf_mode=DoubleRow (2 MACs/cell/cycle:
the PE virtualizes to 128x256, halving matmul instruction count vs bf16).
Weights are host-prescaled by 2^10 (and x by 2^4 for mm1) so the e4m3 lattice
is used well clear of denormals; the gelu activation unscales mm1 by 2^-14.
mm2's 2^10 output scale is never unscaled: the residual stream arrives as
x*2^10 and layernorm is scale-invariant (eps is scaled by 2^20 to match), so
z' = 2^10*z flows straight through the stats and the final scale.

The LN mean term is applied on the HOST: out = wv*rstd*(z - mean) splits into
wv*rstd*z (device) minus (wv*rstd*mean) (a per-token scalar the device ships
as a 1-row tensor), so the device normalize is one DVE multiply per chunk
instead of subtract+multiply.

LN statistics are computed on the PE: an accumulating ones[128,128].T @ zT
matmul (f32r - fp32 truncated to fp22, full PE rate at >=256 cols) sums over
the feature (partition) axis AND broadcasts across all 128 partitions in one
instruction. w1/w2 are fully SBUF-resident in fp8 (4 MB), host-relaid so every
DMA lands as one fat contiguous descriptor per partition (the [m, ko, col]
nesting matches the lhsT access pattern). mm1 is software-pipelined one tile
ahead of mm2/LN, with mm2 accumulation groups interleaved between mm1 groups
so the in-order PE always has an independent group queued behind any group
waiting on DMA or the DVE.
"""

import sys

sys.path.insert(0, "/opt/trn_rl_repo")

import ml_dtypes
import numpy as np

E, K, D = 8, 2, 1024
H = 2 * D  # ffn hidden
B, S = 2, 4096
T = B * S
CT = 512  # max token tile (PSUM bank = 512 f32)
CT_MIN = 128  # remainder-tile granularity
P = 128
KO1 = D // P  # 8  k-chunks for mm1 (4 DoubleRow pairs)
MO1 = H // P  # 16 m-chunks for mm1
KO2 = H // P  # 16 k-chunks for mm2 (8 DoubleRow pairs)
MO2 = D // P  # 8  m-chunks for mm2

SW = 1024.0  # weight prescale (2^10)
SX = 16.0  # x prescale for the fp8 mm1 moving operand (2^4)
FP8_MAX = 240.0  # TRN float8e4 max normal

_kernel_cache = {}


def _tiles_for(C):
    tiles = [(i * CT, CT) for i in range(C // CT)]
    if C % CT:
        tiles.append((C - C % CT, C % CT))
    return tiles


def _build_bass(C, use_b2, use_lng, use_lnb):
    """Build the per-core expert-FFN kernel for capacity C (multiple of CT_MIN)."""
    import concourse.tile as tile
    from concourse import bacc, mybir

    f32 = mybir.dt.float32
    f32r = mybir.dt.float32r
    fp8 = mybir.dt.float8e4
    AF = mybir.ActivationFunctionType
    OP = mybir.AluOpType
    DR = mybir.MatmulPerfMode.DoubleRow

    assert C % CT_MIN == 0
    tiles = _tiles_for(C)
    NT = len(tiles)

    nc = bacc.Bacc("TRN2", target_bir_lowering=False, debug=False)
    # xT/xres ship in per-tile-padded block layouts so each tile's DMA is one
    # contiguous descriptor per partition
    xT_d = nc.dram_tensor("xT", [P, NT, KO1, CT], fp8, kind="ExternalInput")
    xTf_d = nc.dram_tensor("xTf", [P, NT, MO2, CT], f32r, kind="ExternalInput")
    w1_d = nc.dram_tensor("w1", [P, MO1, KO1, P], fp8, kind="ExternalInput")
    w2_d = nc.dram_tensor("w2", [P, MO2, KO2, P], fp8, kind="ExternalInput")
    # host pre-transposes the per-feature constants to [P, mo] so each is a
    # single fat DMA descriptor instead of ~2k 4-byte gathers
    b1_d = nc.dram_tensor("b1", [P, MO1], f32, kind="ExternalInput")
    b2_d = nc.dram_tensor("b2", [P, MO2], f32, kind="ExternalInput")
    lng_d = nc.dram_tensor("ln_g", [P, MO2], f32, kind="ExternalInput")
    lnb_d = nc.dram_tensor("ln_b", [P, MO2], f32, kind="ExternalInput")
    wv_d = nc.dram_tensor("wv", [P, C], f32, kind="ExternalInput")
    out_d = nc.dram_tensor("outT", [D, C], f32, kind="ExternalOutput")
    outS_d = nc.dram_tensor("outS", [1, C], f32, kind="ExternalOutput")

    out_r = out_d.rearrange("(mo p) c -> p mo c", p=P)

    with tile.TileContext(nc) as tc:
        with (
            tc.tile_pool(name="singles", bufs=1) as singles,
            tc.tile_pool(name="xp", bufs=3) as xp,
            tc.tile_pool(name="hp", bufs=2) as hp,
            tc.tile_pool(name="zp", bufs=3) as zp,
            tc.tile_pool(name="zqp", bufs=6) as zqp,
            tc.tile_pool(name="wvp", bufs=3) as wvp,
            tc.tile_pool(name="xrp", bufs=2) as xrp,
            tc.tile_pool(name="stp", bufs=3) as stp,
            tc.tile_pool(name="ocp", bufs=6) as ocp,
            tc.tile_pool(name="psmm", bufs=6, space="PSUM") as psmm,
            tc.tile_pool(name="psst", bufs=1, space="PSUM") as psst,
        ):
            # ---- resident data ----
            # tile 0's xT first (SP ring is FIFO), then w1 in 8 chunks
            # alternating across both HWDGE rings so tile 0's matmuls never
            # outrun the weight stream; w2 is deferred to mid-tile-0 emission
            # so it does not steal HBM bandwidth from w1
            xT_tiles = [xp.tile([P, KO1, CT], fp8, name="xT_sb") for _ in range(3)]
            nc.sync.dma_start(xT_tiles[0][:], xT_d[:, 0])
            ones_f32 = singles.tile([P, P], f32)
            nc.vector.memset(ones_f32[:], 1.0)
            ones_sb = singles.tile([P, P], f32r)
            nc.vector.tensor_copy(ones_sb[:], ones_f32[:])
            eps_sb = singles.tile([P, 1], f32)
            nc.vector.memset(eps_sb[:], 1e-6 * SW * SW)  # eps on 2^10-scaled z
            # each weight chunk is its own tile: the tile dep-tracker is
            # per-tile, so a single resident tile would stall the first matmul
            # until every chunk DMA lands
            w1_ch = [singles.tile([P, 2, KO1, P], fp8, name=f"w1c{i}") for i in range(MO1 // 2)]
            for mq in range(MO1 // 2):
                eng = nc.sync if mq % 2 == 0 else nc.scalar
                eng.dma_start(w1_ch[mq][:], w1_d[:, 2 * mq : 2 * mq + 2])
            b1_sb = singles.tile([P, MO1], f32)
            nc.scalar.dma_start(b1_sb[:], b1_d[:])
            b2_sb = singles.tile([P, MO2], f32)
            nc.scalar.dma_start(b2_sb[:], b2_d[:])
            lng_sb = singles.tile([P, MO2], f32)
            nc.scalar.dma_start(lng_sb[:], lng_d[:])
            lnb_sb = singles.tile([P, MO2], f32)
            nc.scalar.dma_start(lnb_sb[:], lnb_d[:])
            w2_ch = [singles.tile([P, 2, KO2, P], fp8, name=f"w2c{i}") for i in range(MO2 // 2)]

            def emit_w2_chunk(mq):
                nc.scalar.dma_start(w2_ch[mq][:], w2_d[:, 2 * mq : 2 * mq + 2])

            def emit_mm1(t, t0, ct, xT_sb, st, prev_st=None):
                """mm1 (fp8 DoubleRow) + gelu into a fresh hT tile; mm2 groups
                of the previous tile interleave so the PE always has an
                independent accumulation group queued behind a stalled one."""
                hT_sb = hp.tile([P, KO2, CT], fp8, name="hT_sb")
                for m in range(MO1):
                    ps = psmm.tile([P, CT], f32, name="ps_mm")
                    for k in range(KO1 // 2):
                        nc.tensor.matmul(
                            ps[:, :ct],
                            w1_ch[m // 2][:, m % 2, 2 * k : 2 * k + 2, :],
                            xT_sb[:, 2 * k : 2 * k + 2, :ct],
                            start=(k == 0),
                            stop=(k == KO1 // 2 - 1),
                            perf_mode=DR,
                        )
                    # gelu(2^-14 * ps + b1) -> fp8 h (unscaled)
                    nc.scalar.activation(
                        out=hT_sb[:, m, :ct],
                        in_=ps[:, :ct],
                        func=AF.Gelu,
                        bias=b1_sb[:, m : m + 1],
                        scale=1.0 / (SW * SX),
                    )
                    if t == 0 and m in (4, 6, 8, 10):
                        emit_w2_chunk((m - 4) // 2)
                    if m == (6 if t == 0 else 2):
                        nc.scalar.dma_start(st["wv_sb"], wv_d[:, st["ts"]])
                        nc.scalar.dma_start(st["xres"][:, 0:4], xTf_d[:, t, 0:4])
                        nc.scalar.dma_start(st["xres"][:, 4:8], xTf_d[:, t, 4:8])
                    if prev_st is not None and 1 <= m <= MO2:
                        mm2_group(prev_st, m - 1)
                st["hT_sb"] = hT_sb
                return hT_sb

            def mm2_state(t, t0, ct):
                """Allocated ahead of mm1(t); the xres/wv DMAs are emitted
                inside emit_mm1(t) behind a few of tile t's gelus, so they
                start a full tile before mm2(t) consumes them but never steal
                HBM bandwidth from the startup weight stream."""
                ts = slice(t0, t0 + ct)
                wv_sb = wvp.tile([P, CT], f32, name="wv_sb")[:, :ct]
                xres = xrp.tile([P, MO2, CT], f32r, name="xres")
                return {
                    "t": t, "t0": t0, "ct": ct, "ts": ts, "hT_sb": None,
                    "wv_sb": wv_sb, "xres": xres, "norm_prev": None,
                    "zT_sb": zp.tile([P, MO2, CT], f32r, name="zT_sb"),
                    "ps_S": psst.tile([P, CT], f32, name="ps_S"),
                    "ps_Q": psst.tile([P, CT], f32, name="ps_Q"),
                    "pending": [],
                }

            def mm2_stats(st, mo, zc, zq):
                ct = st["ct"]
                nc.tensor.matmul(
                    st["ps_S"][:, :ct], ones_sb[:], zc,
                    start=(mo == 0), stop=(mo == MO2 - 1), skip_group_check=True,
                )
                nc.tensor.matmul(
                    st["ps_Q"][:, :ct], ones_sb[:], zq,
                    start=(mo == 0), stop=(mo == MO2 - 1), skip_group_check=True,
                )

            def mm2_group(st, mo, norm_interleave=True):
                ct = st["ct"]
                ps = psmm.tile([P, CT], f32, name="ps_mm")
                for k in range(KO2 // 2):
                    nc.tensor.matmul(
                        ps[:, :ct],
                        w2_ch[mo // 2][:, mo % 2, 2 * k : 2 * k + 2, :],
                        st["hT_sb"][:, 2 * k : 2 * k + 2, :ct],
                        start=(k == 0),
                        stop=(k == KO2 // 2 - 1),
                        perf_mode=DR,
                    )
                if st["pending"]:
                    mm2_stats(st, *st["pending"].pop())
                zc = st["zT_sb"][:, mo, :ct]
                nc.vector.tensor_tensor(zc, ps[:, :ct], st["xres"][:, mo, :ct], OP.add)
                if use_b2:
                    nc.vector.tensor_scalar_add(zc, zc, b2_sb[:, mo : mo + 1])
                zq = zqp.tile([P, CT], f32r, name="zq")[:, :ct]
                nc.vector.tensor_tensor(zq, zc, zc, OP.mult)
                st["pending"].append((mo, zc, zq))
                # the previous tile's normalize rides one chunk at a time
                # behind this tile's z chain, so the in-order DVE never delays
                # the stats matmuls by a whole tile's worth of normalize work
                if norm_interleave and st["norm_prev"] is not None:
                    ln_norm_chunk(st["norm_prev"], mo)

            def ln_stats(st):
                ct, ts, wv_sb = st["ct"], st["ts"], st["wv_sb"]
                mm2_stats(st, *st["pending"].pop())
                # LN stats: mean/rstd replicated across partitions
                mean_sb = stp.tile([P, CT], f32, name="mean")[:, :ct]
                nc.vector.tensor_scalar_mul(mean_sb, st["ps_S"][:, :ct], 1.0 / D)
                rw_sb = stp.tile([P, CT], f32, name="rw")[:, :ct]
                nc.vector.tensor_scalar_mul(rw_sb, st["ps_Q"][:, :ct], 1.0 / D)
                msq_sb = zqp.tile([P, CT], f32, name="zq")[:, :ct]
                nc.vector.tensor_tensor(msq_sb, mean_sb, mean_sb, OP.mult)
                nc.vector.tensor_tensor(rw_sb, rw_sb, msq_sb, OP.subtract)
                # rw = wv / sqrt(var + eps): sqrt on ACT, fast-reciprocal on DVE
                nc.scalar.activation(
                    out=rw_sb, in_=rw_sb, func=AF.Sqrt, bias=eps_sb[:], scale=1.0,
                )
                nc.vector.reciprocal_approx_fast(rw_sb, rw_sb)
                nc.vector.tensor_tensor(rw_sb, rw_sb, wv_sb, OP.mult)
                # per-token mean term, applied host-side after the combine
                mrw_sb = zqp.tile([P, CT], f32, name="zq")[:, :ct]
                nc.vector.tensor_tensor(mrw_sb, mean_sb, rw_sb, OP.mult)
                nc.scalar.dma_start(outS_d[0:1, ts], mrw_sb[0:1, :ct])
                st["rw_sb"] = rw_sb

            def ln_norm_chunk(st, mo):
                ct = st["ct"]
                oc_last = st.get("oc_last")
                if oc_last is not None:
                    occ = oc_last[:, mo, :]
                else:
                    occ = ocp.tile([P, CT], f32, name="oc")[:, :ct]
                nc.vector.tensor_tensor(occ, st["zT_sb"][:, mo, :ct], st["rw_sb"], OP.mult)
                if use_lng:
                    nc.vector.tensor_scalar_mul(occ, occ, lng_sb[:, mo : mo + 1])
                if use_lnb:
                    lb = stp.tile([P, CT], f32, name="lb")[:, :ct]
                    nc.vector.tensor_scalar_mul(lb, st["wv_sb"], lnb_sb[:, mo : mo + 1])
                    nc.vector.tensor_tensor(occ, occ, lb, OP.add)
                if oc_last is None:
                    nc.sync.dma_start(out_r[:, mo, st["ts"]], occ)
                elif mo == MO2 - 1:
                    nc.sync.dma_start(out_r[:, :, st["ts"]], oc_last[:])

            # xT(t) is consumed only by mm1(t) (residual uses the xres
            # stream); prefetches are queued after the w1 chunks so they never
            # delay the critical w1 stream
            prev_st = None
            for t, (t0, ct) in enumerate(tiles):
                st = mm2_state(t, t0, ct)
                emit_mm1(t, t0, ct, xT_tiles[t % 3], st, prev_st)
                for tn_i in ([1, 2] if t == 0 else [t + 2]):
                    if tn_i < NT:
                        nc.sync.dma_start(xT_tiles[tn_i % 3][:], xT_d[:, tn_i])
                if prev_st is not None:
                    ln_stats(prev_st)
                    st["norm_prev"] = prev_st
                prev_st = st
            ct_last = prev_st["ct"]
            oc_last = singles.tile([P, MO2, ct_last], f32)
            for mo in range(MO2):
                mm2_group(prev_st, mo, norm_interleave=False)
            ln_stats(prev_st)
            if prev_st["norm_prev"] is not None:
                for mo in range(MO2):
                    ln_norm_chunk(prev_st["norm_prev"], mo)
            prev_st["oc_last"] = oc_last
            for mo in range(MO2):
                ln_norm_chunk(prev_st, mo)

    nc.finalize()
    return nc


def _route(x, gate_w):
    """Host gate: top-2 per token + softmax combine weights (matches
    jax.lax.top_k tie-breaking: lower index wins)."""
    xt = x.reshape(-1, D)
    scores = xt.astype(np.float32) @ gate_w.astype(np.float32)  # [T, E]
    e0 = np.argmax(scores, axis=1)
    s0 = scores[np.arange(T), e0]
    masked = scores.copy()
    masked[np.arange(T), e0] = -np.inf
    e1 = np.argmax(masked, axis=1)
    s1 = masked[np.arange(T), e1]
    # softmax over the two selected scores
    mx = np.maximum(s0, s1)
    z0 = np.exp((s0 - mx).astype(np.float64))
    z1 = np.exp((s1 - mx).astype(np.float64))
    den = z0 + z1
    w0 = (z0 / den).astype(np.float32)
    w1 = (z1 / den).astype(np.float32)
    return xt, e0, e1, w0, w1


def _fp8(a, scale):
    return np.clip(a * scale, -FP8_MAX, FP8_MAX).astype(ml_dtypes.float8_e4m3)


def _wlay(w, ko, mo):
    """[ko*P, mo*P] -> [P, mo, ko, P]: partition-contiguous DMA layout that
    matches the lhsT access pattern w_sb[:, m, 2k:2k+2, :]."""
    return np.ascontiguousarray(
        np.asarray(w).reshape(ko, P, mo, P).transpose(1, 2, 0, 3)
    )


def kernel(x, gate_w, w1, b1, w2, b2, ln_g, ln_b):
    from concourse.bass_utils import run_bass_kernel_spmd

    x = np.asarray(x)
    xt, e0, e1, wk0, wk1 = _route(x, np.asarray(gate_w))

    # slot assignment: expert e's token list = tokens with e0==e, then e1==e
    idx_e, wv_e = [], []
    for e in range(E):
        i0 = np.nonzero(e0 == e)[0]
        i1 = np.nonzero(e1 == e)[0]
        idx_e.append(np.concatenate([i0, i1]))
        wv_e.append(np.concatenate([wk0[i0], wk1[i1]]))
    maxn = max(len(i) for i in idx_e)
    C = max(CT_MIN, -(-maxn // CT_MIN) * CT_MIN)
    tiles = _tiles_for(C)
    NT = len(tiles)

    use_b2 = bool(np.any(np.asarray(b2) != 0))
    use_lng = bool(np.any(np.asarray(ln_g) != 1))
    use_lnb = bool(np.any(np.asarray(ln_b) != 0))
    key = (C, use_b2, use_lng, use_lnb)
    if key not in _kernel_cache:
        _kernel_cache[key] = _build_bass(C, use_b2, use_lng, use_lnb)
    nc = _kernel_cache[key]

    in_maps = []
    for e in range(E):
        n = len(idx_e[e])
        xTe = np.zeros((D, C), np.float32)
        xTe[:, :n] = xt[idx_e[e]].T
        wve = np.zeros((C,), np.float32)
        wve[:n] = wv_e[e]
        # per-tile padded block layouts: [P, NT, KO1|MO2, CT]
        x8 = _fp8(xTe, SX).reshape(KO1, P, C)
        xr = (xTe * SW).reshape(MO2, P, C)
        xT_blk = np.zeros((P, NT, KO1, CT), ml_dtypes.float8_e4m3)
        xr_blk = np.zeros((P, NT, MO2, CT), np.float32)
        for t, (t0, ct) in enumerate(tiles):
            xT_blk[:, t, :, :ct] = x8[:, :, t0 : t0 + ct].transpose(1, 0, 2)
            xr_blk[:, t, :, :ct] = xr[:, :, t0 : t0 + ct].transpose(1, 0, 2)
        in_maps.append({
            "xT": xT_blk,
            "xTf": xr_blk,  # 2^10-scaled residual (mm2's never-unscaled scale)
            "w1": _fp8(_wlay(w1[e], KO1, MO1), SW),
            "w2": _fp8(_wlay(w2[e], KO2, MO2), SW),
            "b1": np.ascontiguousarray(np.asarray(b1)[e].reshape(MO1, P).T),
            "b2": np.ascontiguousarray((np.asarray(b2)[e] * SW).reshape(MO2, P).T),
            "ln_g": np.ascontiguousarray(np.asarray(ln_g)[e].reshape(MO2, P).T),
            "ln_b": np.ascontiguousarray(np.asarray(ln_b)[e].reshape(MO2, P).T),
            "wv": np.broadcast_to(wve, (P, C)).copy(),
        })

    res = run_bass_kernel_spmd(nc, in_maps, core_ids=list(range(E)))
    kernel.last_results = res

    # combine: token t's two contributions live at known (expert, slot) pairs
    slot0 = np.empty(T, np.int64)
    slot1 = np.empty(T, np.int64)
    for e in range(E):
        n0 = int(np.sum(e0 == e))
        slot0[e0 == e] = np.arange(n0)
        slot1[e1 == e] = n0 + np.arange(int(np.sum(e1 == e)))
    Y = np.stack([res.results[e]["outT"] for e in range(E)])  # [E, D, C]
    Sm = np.stack([res.results[e]["outS"][0] for e in range(E)])  # [E, C]
    out = Y[e0, :, slot0] + Y[e1, :, slot1]  # [T, D]
    # host-side LN mean term: out -= sum_e (wv*rstd*mean)_e ⊗ ln_g_e
    lng = np.asarray(ln_g, np.float32)
    if use_lng:
        out -= Sm[e0, slot0][:, None] * lng[e0] + Sm[e1, slot1][:, None] * lng[e1]
    else:
        out -= (Sm[e0, slot0] + Sm[e1, slot1])[:, None]
    return out.reshape(x.shape).astype(np.float32)
